# revision 14
# baseline (speedup 1.0000x reference)
"""Trainium2 Bass kernel for a 16-head dense attention block (B=1, S=2048, D=2048).

Sharding: 2 heads per core across 8 cores (tensor parallel on heads).
The reference's (deliberate) transpose(2,3)+reshape before the output
projection makes output rows [h*128:(h+1)*128) depend ONLY on head h, so
per-core outputs are disjoint row blocks -> host-side concat, no collectives.

V2: fp8 DoubleRow compensated matmuls + bf16 attention.
  * QKV projections and out-projection run as 3-pass hi/lo-compensated
    fp8e4m3 DoubleRow matmuls (contraction 256/instr at 0.5 cyc/row =
    25% fewer PE cycles than f32r, half the HBM bytes). hi and lo are
    quantized at the SAME power-2 scale so all 3 passes accumulate into
    one PSUM group with no combine ops.
  * Scale ledger: x*16 (hi/lo), w*1024 (hi/lo) -> psum q/k/v = 16384*true.
    q/k: 1/16384 (and q's 1/sqrt(dh)) folded into bf16 rope constants.
    v: kept raw (16384*v, bf16); the factor rides through AV into O and is
    removed by the oh/ol quantization scale (rt = 1/(1024*sums); 16384/1024
    = 16 = fp8 headroom scale for O). wo*1024 -> out copy scales 1/16384.
  * Rope in rotate-half layout (wq/wk rows pre-permuted host-side to
    [evens; odds]) -> pure partition-offset DVE ops on bf16, no swap DMAs.
  * Scores/AV/sums all bf16 (q/k/v/probs); exp on ACT writes bf16 for
    both heads in one op (scores for both heads share one 2-bank PSUM).
  * Causal masking via affine_select (iota predicate qf >= kp) on the exp
    output - no mask DMA, no mask add. Non-causal masks fall back to the
    mask-load + add path per block. Diagonal blocks fully trimmed
    (off = 128*c; bf16 matmul has no >=256 free-size requirement).
  * Per-g sums for both heads bounce via one DRAM round trip to
    redistribute into per-partition layout for the O^T->O epilogue scale.
"""

import math

import numpy as np

S = 2048
D = 2048
H = 16
DH = 128
N_CORES = 8
HPC = H // N_CORES          # heads per core
NH = HPC * DH               # per-core head rows (256)
P = 128
HALF = 64
QG = 512                    # q/s-group width
NQG = S // QG               # 4
NKT = S // P                # 16 k tiles
NDT = D // P                # 16 d tiles

SKIP, NOMASK, CAUSAL, MASKED = 0, 1, 2, 3

_CACHE = {}


def _build(block_kind):
    """block_kind: tuple of NQG tuples of NKT (kind, off) pairs."""
    import os
    import concourse.tile as tile
    from concourse import bacc, mybir

    B = lambda k, d: int(os.environ.get(k, d))
    _os = os
    f32 = mybir.dt.float32
    f32r = mybir.dt.float32r
    bf16 = mybir.dt.bfloat16
    fp8 = mybir.dt.float8e4
    EXP = mybir.ActivationFunctionType.Exp
    COPY = mybir.ActivationFunctionType.Copy
    DR = mybir.MatmulPerfMode.DoubleRow
    MUL = mybir.AluOpType.mult
    SUB = mybir.AluOpType.subtract

    nc = bacc.Bacc("TRN2", target_bir_lowering=False, debug=False,
                   num_devices=N_CORES)

    xh = nc.dram_tensor("xh", [P, NDT, S], fp8, kind="ExternalInput").ap()
    xl = nc.dram_tensor("xl", [P, NDT, S], fp8, kind="ExternalInput").ap()
    w_in = {}
    for kind in ("q", "k", "v"):
        for part in ("h", "l"):
            nm = f"w{kind}{part}"
            shape = [P, NDT, NH] if kind == "v" else [HPC, P, NDT, P]
            w_in[nm] = nc.dram_tensor(nm, shape, fp8,
                                      kind="ExternalInput").ap()
    woh = nc.dram_tensor("woh", [P, NKT, D], fp8, kind="ExternalInput").ap()
    wol = nc.dram_tensor("wol", [P, NKT, D], fp8, kind="ExternalInput").ap()
    maskT = nc.dram_tensor("maskT", [S, S], f32, kind="ExternalInput").ap()
    cq = nc.dram_tensor("cq", [DH, S], bf16, kind="ExternalInput").ap()
    sq = nc.dram_tensor("sq", [DH, S], bf16, kind="ExternalInput").ap()
    ck = nc.dram_tensor("ck", [DH, S], bf16, kind="ExternalInput").ap()
    sk = nc.dram_tensor("sk", [DH, S], bf16, kind="ExternalInput").ap()
    ones = nc.dram_tensor("ones", [P, 1], bf16, kind="ExternalInput").ap()
    cmask = nc.dram_tensor("cmask", [P, HPC, QG], bf16,
                           kind="ExternalInput").ap()
    out = nc.dram_tensor("out", [NH, D], f32, kind="ExternalOutput").ap()

    with tile.TileContext(nc) as tc:
        with tc.tile_pool(name="consts", bufs=1) as consts, \
             tc.tile_pool(name="qkv", bufs=1) as qkv, \
             tc.tile_pool(name="att_p", bufs=1) as att_p, \
             tc.tile_pool(name="wo_sb", bufs=B("BW", 1)) as wo_sb, \
             tc.tile_pool(name="proj_sb", bufs=1) as proj_sb, \
             tc.tile_pool(name="rope_sb", bufs=B("BR", 4)) as rope_sb, \
             tc.tile_pool(name="att_sb", bufs=B("BA", 4)) as att_sb, \
             tc.tile_pool(name="r_sb", bufs=B("BRS", 2)) as r_sb, \
             tc.tile_pool(name="ps_proj", bufs=B("BP", 3), space="PSUM") as ps_proj, \
             tc.tile_pool(name="ps_sc", bufs=B("BS", 1), space="PSUM") as ps_sc, \
             tc.tile_pool(name="ps_o", bufs=B("BO", 1), space="PSUM") as ps_o, \
             tc.tile_pool(name="ps_sum", bufs=B("BSM", 1), space="PSUM") as ps_sum:

            ones_t = consts.tile([P, 1], bf16, tag="ones")
            nc.scalar.dma_start(ones_t[:], ones[:])
            cm_t = consts.tile([P, HPC, QG], bf16, tag="cmask")
            nc.scalar.dma_start(cm_t[:], cmask[:])
            # warm the Exp activation table off the critical path
            warm = consts.tile([1, 1], bf16, tag="warm")
            nc.scalar.activation(warm[:], ones_t[0:1, 0:1], EXP)

            qt = [qkv.tile([P, S], bf16, tag=f"qt{h}", name=f"qt{h}")
                  for h in range(HPC)]
            kt_ = [qkv.tile([P, S], bf16, tag=f"kt{h}", name=f"kt{h}")
                   for h in range(HPC)]
            vt = qkv.tile([P, NKT, NH], bf16, tag="v")   # [k-part, ktile, n]

            oh_sb = [att_p.tile([P, NKT, P], fp8, tag=f"oh{h}",
                                name=f"oh{h}") for h in range(HPC)]
            ol_sb = [att_p.tile([P, NKT, P], fp8, tag=f"ol{h}",
                                name=f"ol{h}") for h in range(HPC)]
            rt = att_p.tile([P, HPC, NKT], f32, tag="rt", name="rt")
            maskT_v = maskT.rearrange("(t p) s -> t p s", p=P)

            # ---- attention block emitters (drained between QKV groups) --
            # probs are the STATIONARY operand of both the AV and the sums
            # matmuls: AV emits O directly in [s, dh] layout (no epilogue
            # PE transposes) and the sums matmul has output free-size 1
            # (~free in the PE cost model vs a full moving-width pass).
            # exp runs once per (g, kt) over both heads' scores ([P,2,nw]).
            # PSUM accumulation start=True zeroes the tile's whole 2KB zero
            # region, so emit exactly ONE start (first matmul into the
            # tile) and ONE stop (last matmul); per-chunk first touches
            # overwrite via pending-zero.
            gstate = {}

            def _att_begin(g):
                kinds = block_kind[g]
                active = [kt for kt in range(NKT) if kinds[kt][0] != SKIP]
                first = active[0]
                offs = {kt: (0 if kt == first else kinds[kt][1])
                        for kt in active}
                gstate[g] = dict(
                    kinds=kinds, active=active, first=first, offs=offs,
                    last_kt=active[-1], po_first=[True] * HPC,
                    pss_first=[True],
                    pss=ps_sum.tile([P, HPC, 4], f32, tag="pss",
                                    name="pss"),
                    po=[ps_o.tile([P, 4, P], f32, tag=f"po{h}",
                                  name=f"po{h}") for h in range(HPC)])

            def _att_block(g, kt):
                st_ = gstate[g]
                bkind, off = st_["kinds"][kt][0], st_["offs"][kt]
                po, pss = st_["po"], st_["pss"]
                nw = QG - off
                qsl = slice(g * QG + off, (g + 1) * QG)
                osl = slice(off, QG)
                ksl = slice(kt * P, (kt + 1) * P)
                pt2 = att_sb.tile([P, HPC, QG], bf16, tag="pt", name="pt")
                ps2 = ps_sc.tile([P, HPC, QG], f32, tag="sc", name="sc")
                for h in range(HPC):
                    nc.tensor.matmul(ps2[:, h, osl], kt_[h][:, ksl],
                                     qt[h][:, qsl], start=True, stop=True)
                if bkind == MASKED:
                    mt = att_sb.tile([P, QG], f32, tag="mask", name="mt")
                    nc.sync.dma_start(mt[:, osl], maskT_v[kt][:, qsl])
                    for h in range(HPC):
                        sm = att_sb.tile([P, QG], f32, tag="sm", name="sm")
                        nc.vector.tensor_add(sm[:, osl], ps2[:, h, osl],
                                             mt[:, osl])
                        nc.scalar.activation(pt2[:, h, osl], sm[:, osl],
                                             EXP)
                else:
                    nc.scalar.activation(pt2[:, :, osl], ps2[:, :, osl],
                                         EXP)
                    if bkind == CAUSAL:
                        # zero where qf < kp (off == 128*c): the 0/1
                        # keep-pattern is block-independent
                        nc.vector.tensor_mul(pt2[:, :, osl],
                                             pt2[:, :, osl],
                                             cm_t[:, :, 0:nw])
                for h in range(HPC):
                    for c in range(off // P, 4):
                        csl = slice(c * P, (c + 1) * P)
                        sp = (kt == st_["last_kt"] and c == 3)
                        nc.tensor.matmul(
                            po[h][:, c, :], pt2[:, h, csl],
                            vt[:, kt, h * P:(h + 1) * P],
                            start=st_["po_first"][h], stop=sp)
                        st_["po_first"][h] = False
                        nc.tensor.matmul(
                            pss[:, h, c:c + 1], pt2[:, h, csl], ones_t[:],
                            start=st_["pss_first"][0],
                            stop=(sp and h == HPC - 1))
                        st_["pss_first"][0] = False

            def _att_epilogue(g, h):
                # sums land per-partition (p <-> q = jt*128+p): reciprocal
                # and oh/ol quantization read PSUM directly.
                st_ = gstate[g]
                g4 = slice(g * 4, g * 4 + 4)
                nc.vector.reciprocal(rt[:, h, g4], st_["pss"][:, h, 0:4])
                for c in range(4):
                    jt = g * 4 + c
                    rsc = rt[:, h, jt:jt + 1]
                    nc.vector.tensor_scalar_mul(oh_sb[h][:, jt],
                                                st_["po"][h][:, c, :], rsc)
                    nc.vector.scalar_tensor_tensor(
                        ol_sb[h][:, jt], st_["po"][h][:, c, :], rsc,
                        oh_sb[h][:, jt], op0=MUL, op1=SUB)

            ready = []          # FIFO of (g, kt) attention blocks
            PACE = B("PACE", 2)

            def _drain(n):
                while n > 0 and ready:
                    g, kt = ready.pop(0)
                    _att_block(g, kt)
                    if kt == gstate[g]["last_kt"]:
                        for h in range(HPC):
                            _att_epilogue(g, h)
                    n -= 1

            # ------------- fused QKV + attention g-pipeline --------------
            if True:
                # DMA_ENGINES serves queues round-robin but is serial overall
                # in the cost model, so everything phase-1-critical goes on
                # the SP queue in exact dependency order: wqh, xsh(g0), wql,
                # xsl(g0), rope q-consts, wk, rope k-consts, wv, xs(g1..3).
                NCH = 4                       # x chunks per group
                CW = NDT // NCH               # d-tiles per chunk
                wts = {}

                def _load_w(nm, h):
                    """per-head half of a w tensor: [P, NDT, P]"""
                    t = proj_sb.tile([P, NDT, P], fp8, tag=f"{nm}{h}",
                                     name=f"{nm}{h}")
                    nc.sync.dma_start(t[:], w_in[nm][h])
                    wts[(nm, h)] = t

                class _XS:
                    """xs[:, us, ...] pair-slicing across 4 chunk tiles."""
                    def __init__(self, chunks):
                        self.chunks = chunks
                    def __getitem__(self, idx):
                        us = idx[1]
                        c, lo = us.start // CW, us.start % CW
                        rest = idx[2:]
                        return self.chunks[c][
                            (slice(None), slice(lo, lo + 2)) + rest]

                def _load_xs(part, src, sl, eng=None):
                    eng = eng or nc.sync
                    chunks = []
                    for c in range(NCH):
                        cs = slice(c * CW, (c + 1) * CW)
                        t = proj_sb.tile([P, CW, QG], fp8,
                                         tag=f"xs{part}{c}", bufs=2,
                                         name=f"xs{part}{c}")
                        eng.dma_start(t[:], src[:, cs, sl])
                        chunks.append(t)
                    return _XS(chunks)

                rope_t = {}

                def _load_rope(*names):
                    for nm in names:
                        src = {"cq": cq, "sq": sq, "ck": ck, "sk": sk}[nm]
                        t = proj_sb.tile([DH, S], bf16, tag=nm, name=nm)
                        nc.sync.dma_start(t[:], src[:])
                        rope_t[nm] = t

                def _load_wv(nm):
                    t = proj_sb.tile([P, NDT, NH], fp8, tag=nm, name=nm)
                    nc.sync.dma_start(t[:], w_in[nm][:])
                    wts[nm] = t

                sl0 = slice(0, QG)
                _load_w("wqh", 0)
                xsh0 = _load_xs("h", xh, sl0)
                _load_w("wql", 0)
                xsl0 = _load_xs("l", xl, sl0)
                for nm in ("wqh", "wql", "wkh", "wkl"):
                    for h in range(HPC):
                        if (nm, h) not in wts:
                            _load_w(nm, h)
                _load_wv("wvh")
                _load_wv("wvl")
                _load_rope("cq", "sq", "ck", "sk")

                for g in range(NQG):
                    sl = slice(g * QG, (g + 1) * QG)
                    if g == 0:
                        xsh, xsl = xsh0, xsl0
                    else:
                        xsh = _load_xs("h", xh, sl)
                        xsl = _load_xs("l", xl, sl)

                    kinds_order = (("q", qt, "cq", "sq"),
                                   ("k", kt_, "ck", "sk"))
                    for kind, dst, cn, sn in kinds_order:
                        for h in range(HPC):
                            wh_t = wts[(f"w{kind}h", h)]
                            wl_t = wts[(f"w{kind}l", h)]
                            ps = ps_proj.tile([P, QG], f32, tag="pq",
                                              name="pq")
                            n = 0
                            for mv, st in ((xsh, wh_t), (xsh, wl_t),
                                           (xsl, wh_t)):
                                for u in range(NDT // 2):
                                    us = slice(2 * u, 2 * u + 2)
                                    nc.tensor.matmul(
                                        ps[:], st[:, us], mv[:, us],
                                        start=(n == 0), stop=(n == 23),
                                        perf_mode=DR)
                                    n += 1
                            # psum (16384*raw) -> bf16 raw on ACT
                            raw = rope_sb.tile([P, QG], bf16, tag="raw",
                                               name="raw")
                            nc.scalar.copy(raw[:], ps[:])
                            # rope (quadrant-interleaved pair layout):
                            # out = raw*c + shuffle(raw)*s, where the
                            # shuffle swaps slots t<->t+16 in each quadrant
                            m = dst[h]
                            sw = rope_sb.tile([P, QG], bf16, tag="sw",
                                              name="sw")
                            nc.vector.stream_shuffle(
                                sw[:], raw[:],
                                [(i + 16) % 32 for i in range(32)])
                            nc.vector.tensor_mul(sw[:], sw[:],
                                                 rope_t[sn][:, sl])
                            nc.vector.tensor_mul(m[:, sl], raw[:],
                                                 rope_t[cn][:, sl])
                            nc.vector.tensor_add(m[:, sl], m[:, sl], sw[:])
                            _drain(PACE)
                    # attention blocks of g needing only previously
                    # computed v tiles can start as soon as q(g) is roped
                    kinds_g = block_kind[g]
                    act_g = [kt for kt in range(NKT)
                             if kinds_g[kt][0] != SKIP]
                    if act_g:
                        _att_begin(g)
                        ready.extend((g, kt) for kt in act_g
                                     if kt < 4 * g)
                    wvh_t, wvl_t = wts["wvh"], wts["wvl"]
                    for st_i in range(g * 4, g * 4 + 4):
                        lsl = slice((st_i % 4) * P, (st_i % 4) * P + P)
                        ps = ps_proj.tile([P, QG], f32, tag="pq",
                                          name="pq")
                        n = 0
                        for mv, stw in ((xsh, wvh_t), (xsh, wvl_t),
                                        (xsl, wvh_t)):
                            for u in range(NDT // 2):
                                us = slice(2 * u, 2 * u + 2)
                                nc.tensor.matmul(
                                    ps[:, 0:NH], mv[:, us, lsl],
                                    stw[:, us],
                                    start=(n == 0), stop=(n == 23),
                                    perf_mode=DR)
                                n += 1
                        nc.scalar.copy(vt[:, st_i], ps[:, 0:NH])
                        _drain(PACE)
                    ready.extend((g, kt) for kt in act_g if kt >= 4 * g)

            # wo loads: issued on the SP queue AFTER all x loads so they
            # never delay the x feed; DMA has slack during attention.
            wo_t = []
            for mg in range(NQG):
                msl = slice(mg * QG, (mg + 1) * QG)
                wh_t = wo_sb.tile([P, NKT, QG], fp8, tag=f"woh{mg}",
                                  name=f"woh{mg}")
                nc.sync.dma_start(wh_t[:], woh[:, :, msl])
                wl_t = wo_sb.tile([P, NKT, QG], fp8, tag=f"wol{mg}",
                                  name=f"wol{mg}")
                nc.sync.dma_start(wl_t[:], wol[:, :, msl])
                wo_t.append((wh_t, wl_t))

            # ------------- tail: remaining attention + out-projection ----
            def _proj(h, mg, csl_):
                width = csl_.stop - csl_.start
                wh_t, wl_t = wo_t[mg]
                psr = ps_proj.tile([P, QG], f32, tag="pq", name="pq")
                n = 0
                for st, mv in ((oh_sb[h], wh_t), (ol_sb[h], wh_t),
                               (oh_sb[h], wl_t)):
                    for u in range(NKT // 2):
                        us = slice(2 * u, 2 * u + 2)
                        nc.tensor.matmul(
                            psr[:, 0:width], st[:, us], mv[:, us, csl_],
                            start=(n == 0), stop=(n == 23),
                            perf_mode=DR)
                        n += 1
                rs = r_sb.tile([P, QG], f32, tag="rs", name="rs")
                nc.scalar.activation(rs[:, 0:width], psr[:, 0:width],
                                     COPY, scale=1.0 / 16384.0)
                msl = slice(mg * QG + csl_.start,
                            mg * QG + csl_.stop)
                nc.sync.dma_start(out[h * P:(h + 1) * P, msl],
                                  rs[:, 0:width])

            proj_jobs = [(h, mg) for h in range(HPC) for mg in range(NQG)]
            while ready:
                _drain(1)
            for i, (h, mg) in enumerate(proj_jobs):
                if i == len(proj_jobs) - 1:
                    # split the last group so the final copy+store tail
                    # is half as long
                    _proj(h, mg, slice(0, QG // 2))
                    _proj(h, mg, slice(QG // 2, QG))
                else:
                    _proj(h, mg, slice(0, QG))

    nc.compile()
    return nc


def _classify_mask(maskT):
    """Per (g, kt) block of maskT [k, q]: (SKIP|NOMASK|CAUSAL|MASKED, off)."""
    kinds = []
    qi = np.arange(QG)
    ki = np.arange(P)
    for g in range(NQG):
        row = []
        for kt in range(NKT):
            blk = maskT[kt * P:(kt + 1) * P, g * QG:(g + 1) * QG]
            if np.all(blk <= -1e5):
                row.append((SKIP, 0))
                continue
            if not blk.any():
                row.append((NOMASK, 0))
                continue
            c = kt - 4 * g
            if 0 <= c <= 3:
                keep = (g * QG + qi[None, :]) >= (kt * P + ki[:, None])
                if (np.all((blk == 0) == keep)
                        and np.all(blk[~keep] <= -1e5)):
                    row.append((CAUSAL, P * c))
                    continue
            off = 0
            while (off + P <= QG - P
                   and np.all(blk[:, off:off + P] <= -1e5)):
                off += P
            row.append((MASKED, off))
        kinds.append(tuple(row))
    return tuple(kinds)


def _get_nc(block_kind):
    key = ("nc", block_kind)
    if key not in _CACHE:
        _CACHE[key] = _build(block_kind)
    return _CACHE[key]


def _prep_inputs(x, freqs_cos, freqs_sin, mask, wq, wk, wv, wo):
    import ml_dtypes
    f = np.float32
    E4 = ml_dtypes.float8_e4m3
    BF = ml_dtypes.bfloat16

    x = np.asarray(x, f).reshape(S, D)
    mask = np.asarray(mask, f).reshape(S, S)
    wq, wk, wv, wo = (np.asarray(w, f) for w in (wq, wk, wv, wo))
    cos = np.asarray(freqs_cos, f)
    sin = np.asarray(freqs_sin, f)

    def split8(a):
        hi = a.astype(E4)
        lo = (a - hi.astype(f)).astype(E4)
        return hi, lo

    def to_pts(a2d, scale):
        """[R, C] f32 -> hi/lo fp8 in [P, R//P, C] layout."""
        a = a2d * scale
        hi, lo = split8(a)
        def lay(t):
            return np.ascontiguousarray(
                t.reshape(R // P, P, -1).transpose(1, 0, 2))
        R = a2d.shape[0]
        return lay(hi), lay(lo)

    xT = np.ascontiguousarray(x.T)                     # [D, S]
    xh_, xl_ = to_pts(xT, 16.0)

    # quadrant-interleaved rope layout for q/k head dims: quadrant q's
    # 32 slots hold even dims 2*(16q+t) for t<16, then their odd partners.
    # The t<->t+16 swap is then a DVE stream_shuffle (intra-quadrant).
    perm = np.empty(DH, np.int64)
    r = np.arange(DH)
    quad, t = r // 32, r % 32
    perm[:] = np.where(t < 16, 32 * quad + 2 * (t % 16),
                       32 * quad + 2 * (t % 16) + 1)
    jidx = 16 * quad + (t % 16)        # freq index per row
    maskT = np.ascontiguousarray(mask.T)

    C2 = cos.T[jidx]                                   # [128, S]
    S2 = np.where((t < 16)[:, None], -sin.T[jidx], sin.T[jidx])
    sc_q = (1.0 / 16384.0) / math.sqrt(DH)
    sc_k = 1.0 / 16384.0

    woT = np.ascontiguousarray(wo.T)                   # [j, m]
    woh_, wol_ = to_pts(woT, 1024.0)

    common = {
        "xh": xh_, "xl": xl_, "maskT": maskT,
        "woh": woh_, "wol": wol_,
        "cq": np.ascontiguousarray(C2 * sc_q).astype(BF),
        "sq": np.ascontiguousarray(S2 * sc_q).astype(BF),
        "ck": np.ascontiguousarray(C2 * sc_k).astype(BF),
        "sk": np.ascontiguousarray(S2 * sc_k).astype(BF),
        # sums moving vector: folds the 2^10 fp8-headroom scale so the
        # reciprocal of the raw PSUM sums is directly the oh/ol scale
        "ones": np.full((P, 1), 1024.0, BF),
        "cmask": np.broadcast_to(
            (np.arange(QG)[None, None, :]
             >= np.arange(P)[:, None, None]), (P, HPC, QG)).astype(BF),
    }
    in_maps = []
    for c in range(N_CORES):
        rows = np.arange(c * NH, (c + 1) * NH)
        # permuted row order for q/k: per head [evens; odds]
        rows_pm = np.concatenate(
            [c * NH + h * DH + perm for h in range(HPC)])
        m = dict(common)
        for kind, w in (("q", wq), ("k", wk), ("v", wv)):
            r = rows_pm if kind in ("q", "k") else rows
            wT = np.ascontiguousarray(w[r].T)          # [D, NH]
            hi, lo = to_pts(wT, 1024.0)                # [P, NDT, NH]
            if kind != "v":
                # per-head-major: [HPC, P, NDT, P]
                hi = np.ascontiguousarray(
                    hi.reshape(P, NDT, HPC, P).transpose(2, 0, 1, 3))
                lo = np.ascontiguousarray(
                    lo.reshape(P, NDT, HPC, P).transpose(2, 0, 1, 3))
            m[f"w{kind}h"] = hi
            m[f"w{kind}l"] = lo
        in_maps.append(m)
    return in_maps


def kernel(x, freqs_cos, freqs_sin, mask, wq, wk, wv, wo, start_pos):
    from concourse.bass_utils import run_bass_kernel_spmd

    in_maps = _prep_inputs(x, freqs_cos, freqs_sin, mask, wq, wk, wv, wo)
    nc = _get_nc(_classify_mask(in_maps[0]["maskT"]))
    res = run_bass_kernel_spmd(nc, in_maps, core_ids=list(range(N_CORES)))
    full = np.concatenate([res.results[c]["out"] for c in range(N_CORES)],
                          axis=0)
    return full.reshape(1, S, D).astype(np.float32)



# revision 22
# speedup vs baseline: 1.0208x; 1.0208x over previous
"""Trainium2 Bass kernel for a 16-head dense attention block (B=1, S=2048, D=2048).

Sharding: 2 heads per core across 8 cores (tensor parallel on heads).
The reference's (deliberate) transpose(2,3)+reshape before the output
projection makes output rows [h*128:(h+1)*128) depend ONLY on head h, so
per-core outputs are disjoint row blocks -> host-side concat, no collectives.

V2: fp8 DoubleRow compensated matmuls + bf16 attention.
  * QKV projections and out-projection run as 3-pass hi/lo-compensated
    fp8e4m3 DoubleRow matmuls (contraction 256/instr at 0.5 cyc/row =
    25% fewer PE cycles than f32r, half the HBM bytes). hi and lo are
    quantized at the SAME power-2 scale so all 3 passes accumulate into
    one PSUM group with no combine ops.
  * Scale ledger: x*16 (hi/lo), w*1024 (hi/lo) -> psum q/k/v = 16384*true.
    q/k: 1/16384 (and q's 1/sqrt(dh)) folded into bf16 rope constants.
    v: kept raw (16384*v, bf16); the factor rides through AV into O and is
    removed by the oh/ol quantization scale (rt = 1/(1024*sums); 16384/1024
    = 16 = fp8 headroom scale for O). wo*1024 -> out copy scales 1/16384.
  * Rope in rotate-half layout (wq/wk rows pre-permuted host-side to
    [evens; odds]) -> pure partition-offset DVE ops on bf16, no swap DMAs.
  * Scores/AV/sums all bf16 (q/k/v/probs); exp on ACT writes bf16 for
    both heads in one op (scores for both heads share one 2-bank PSUM).
  * Causal masking via affine_select (iota predicate qf >= kp) on the exp
    output - no mask DMA, no mask add. Non-causal masks fall back to the
    mask-load + add path per block. Diagonal blocks fully trimmed
    (off = 128*c; bf16 matmul has no >=256 free-size requirement).
  * Per-g sums for both heads bounce via one DRAM round trip to
    redistribute into per-partition layout for the O^T->O epilogue scale.
"""

import math

import numpy as np

S = 2048
D = 2048
H = 16
DH = 128
N_CORES = 8
HPC = H // N_CORES          # heads per core
NH = HPC * DH               # per-core head rows (256)
P = 128
HALF = 64
QG = 512                    # q/s-group width
NQG = S // QG               # 4
NKT = S // P                # 16 k tiles
NDT = D // P                # 16 d tiles

SKIP, NOMASK, CAUSAL, MASKED = 0, 1, 2, 3

_CACHE = {}


def _build(block_kind):
    """block_kind: tuple of NQG tuples of NKT (kind, off) pairs."""
    import os
    import concourse.tile as tile
    from concourse import bacc, mybir

    B = lambda k, d: int(os.environ.get(k, d))
    _os = os
    f32 = mybir.dt.float32
    f32r = mybir.dt.float32r
    bf16 = mybir.dt.bfloat16
    fp8 = mybir.dt.float8e4
    EXP = mybir.ActivationFunctionType.Exp
    COPY = mybir.ActivationFunctionType.Copy
    DR = mybir.MatmulPerfMode.DoubleRow
    MUL = mybir.AluOpType.mult
    SUB = mybir.AluOpType.subtract

    nc = bacc.Bacc("TRN2", target_bir_lowering=False, debug=False,
                   num_devices=N_CORES)

    xh = nc.dram_tensor("xh", [P, NDT, S], fp8, kind="ExternalInput").ap()
    xl = nc.dram_tensor("xl", [P, NDT, S], fp8, kind="ExternalInput").ap()
    w_in = {}
    for kind in ("q", "k", "v"):
        for part in ("h", "l"):
            nm = f"w{kind}{part}"
            shape = [P, NDT, NH] if kind == "v" else [HPC, P, NDT, P]
            w_in[nm] = nc.dram_tensor(nm, shape, fp8,
                                      kind="ExternalInput").ap()
    woh = nc.dram_tensor("woh", [P, NKT, D], fp8, kind="ExternalInput").ap()
    wol = nc.dram_tensor("wol", [P, NKT, D], fp8, kind="ExternalInput").ap()
    maskT = nc.dram_tensor("maskT", [S, S], f32, kind="ExternalInput").ap()
    cq = nc.dram_tensor("cq", [DH, S], bf16, kind="ExternalInput").ap()
    sq = nc.dram_tensor("sq", [DH, S], bf16, kind="ExternalInput").ap()
    ck = nc.dram_tensor("ck", [DH, S], bf16, kind="ExternalInput").ap()
    sk = nc.dram_tensor("sk", [DH, S], bf16, kind="ExternalInput").ap()
    ones = nc.dram_tensor("ones", [P, 1], bf16, kind="ExternalInput").ap()
    cmask = nc.dram_tensor("cmask", [P, HPC, QG], bf16,
                           kind="ExternalInput").ap()
    out = nc.dram_tensor("out", [NH, D], f32, kind="ExternalOutput").ap()

    with tile.TileContext(nc) as tc:
        with tc.tile_pool(name="consts", bufs=1) as consts, \
             tc.tile_pool(name="qkv", bufs=1) as qkv, \
             tc.tile_pool(name="att_p", bufs=1) as att_p, \
             tc.tile_pool(name="wo_sb", bufs=B("BW", 1)) as wo_sb, \
             tc.tile_pool(name="proj_sb", bufs=1) as proj_sb, \
             tc.tile_pool(name="rope_sb", bufs=B("BR", 4)) as rope_sb, \
             tc.tile_pool(name="att_sb", bufs=B("BA", 4)) as att_sb, \
             tc.tile_pool(name="r_sb", bufs=B("BRS", 4)) as r_sb, \
             tc.tile_pool(name="ps_proj", bufs=B("BP", 3), space="PSUM") as ps_proj, \
             tc.tile_pool(name="ps_sc", bufs=B("BS", 1), space="PSUM") as ps_sc, \
             tc.tile_pool(name="ps_o", bufs=B("BO", 1), space="PSUM") as ps_o, \
             tc.tile_pool(name="ps_sum", bufs=B("BSM", 1), space="PSUM") as ps_sum:

            ones_t = consts.tile([P, 1], bf16, tag="ones")
            nc.scalar.dma_start(ones_t[:], ones[:])
            cm_t = consts.tile([P, HPC, QG], bf16, tag="cmask")
            nc.scalar.dma_start(cm_t[:], cmask[:])
            # warm the Exp activation table off the critical path
            warm = consts.tile([1, 1], bf16, tag="warm")
            nc.scalar.activation(warm[:], ones_t[0:1, 0:1], EXP)

            qt = [qkv.tile([P, S], bf16, tag=f"qt{h}", name=f"qt{h}")
                  for h in range(HPC)]
            kt_ = [qkv.tile([P, S], bf16, tag=f"kt{h}", name=f"kt{h}")
                   for h in range(HPC)]
            vt = qkv.tile([P, NKT, NH], bf16, tag="v")   # [k-part, ktile, n]

            oh_sb = [att_p.tile([P, NKT, P], fp8, tag=f"oh{h}",
                                name=f"oh{h}") for h in range(HPC)]
            ol_sb = [att_p.tile([P, NKT, P], fp8, tag=f"ol{h}",
                                name=f"ol{h}") for h in range(HPC)]
            rt = att_p.tile([P, HPC, NKT], f32, tag="rt", name="rt")
            maskT_v = maskT.rearrange("(t p) s -> t p s", p=P)

            # ---- attention block emitters (drained between QKV groups) --
            # probs are the STATIONARY operand of both the AV and the sums
            # matmuls: AV emits O directly in [s, dh] layout (no epilogue
            # PE transposes) and the sums matmul has output free-size 1
            # (~free in the PE cost model vs a full moving-width pass).
            # exp runs once per (g, kt) over both heads' scores ([P,2,nw]).
            # PSUM accumulation start=True zeroes the tile's whole 2KB zero
            # region, so emit exactly ONE start (first matmul into the
            # tile) and ONE stop (last matmul); per-chunk first touches
            # overwrite via pending-zero.
            gstate = {}

            def _att_begin(g):
                kinds = block_kind[g]
                active = [kt for kt in range(NKT) if kinds[kt][0] != SKIP]
                first = active[0]
                offs = {kt: (0 if kt == first else kinds[kt][1])
                        for kt in active}
                gstate[g] = dict(
                    kinds=kinds, active=active, first=first, offs=offs,
                    last_kt=active[-1], po_first=[True] * HPC,
                    pss_first=[True],
                    pss=ps_sum.tile([P, HPC, 4], f32, tag="pss",
                                    name="pss"),
                    po=[ps_o.tile([P, 4, P], f32, tag=f"po{h}",
                                  name=f"po{h}") for h in range(HPC)])

            def _att_block(g, kt):
                st_ = gstate[g]
                bkind, off = st_["kinds"][kt][0], st_["offs"][kt]
                po, pss = st_["po"], st_["pss"]
                nw = QG - off
                qsl = slice(g * QG + off, (g + 1) * QG)
                osl = slice(off, QG)
                ksl = slice(kt * P, (kt + 1) * P)
                pt2 = att_sb.tile([P, HPC, QG], bf16, tag="pt", name="pt")
                ps2 = ps_sc.tile([P, HPC, QG], f32, tag="sc", name="sc")
                for h in range(HPC):
                    nc.tensor.matmul(ps2[:, h, osl], kt_[h][:, ksl],
                                     qt[h][:, qsl], start=True, stop=True)
                if bkind == MASKED:
                    mt = att_sb.tile([P, QG], f32, tag="mask", name="mt")
                    nc.sync.dma_start(mt[:, osl], maskT_v[kt][:, qsl])
                    for h in range(HPC):
                        sm = att_sb.tile([P, QG], f32, tag="sm", name="sm")
                        nc.vector.tensor_add(sm[:, osl], ps2[:, h, osl],
                                             mt[:, osl])
                        nc.scalar.activation(pt2[:, h, osl], sm[:, osl],
                                             EXP)
                else:
                    nc.scalar.activation(pt2[:, :, osl], ps2[:, :, osl],
                                         EXP)
                    if bkind == CAUSAL:
                        # zero where qf < kp (off == 128*c): the 0/1
                        # keep-pattern is block-independent
                        nc.vector.tensor_mul(pt2[:, :, osl],
                                             pt2[:, :, osl],
                                             cm_t[:, :, 0:nw])
                for h in range(HPC):
                    for c in range(off // P, 4):
                        csl = slice(c * P, (c + 1) * P)
                        sp = (kt == st_["last_kt"] and c == 3)
                        nc.tensor.matmul(
                            po[h][:, c, :], pt2[:, h, csl],
                            vt[:, kt, h * P:(h + 1) * P],
                            start=st_["po_first"][h], stop=sp)
                        st_["po_first"][h] = False
                        nc.tensor.matmul(
                            pss[:, h, c:c + 1], pt2[:, h, csl], ones_t[:],
                            start=st_["pss_first"][0],
                            stop=(sp and h == HPC - 1))
                        st_["pss_first"][0] = False

            def _att_epilogue(g, h):
                # sums land per-partition (p <-> q = jt*128+p): reciprocal
                # and oh/ol quantization read PSUM directly.
                st_ = gstate[g]
                g4 = slice(g * 4, g * 4 + 4)
                nc.vector.reciprocal(rt[:, h, g4], st_["pss"][:, h, 0:4])
                for c in range(4):
                    jt = g * 4 + c
                    rsc = rt[:, h, jt:jt + 1]
                    nc.vector.tensor_scalar_mul(oh_sb[h][:, jt],
                                                st_["po"][h][:, c, :], rsc)
                    nc.vector.scalar_tensor_tensor(
                        ol_sb[h][:, jt], st_["po"][h][:, c, :], rsc,
                        oh_sb[h][:, jt], op0=MUL, op1=SUB)

            ready = []          # FIFO of (g, kt) attention blocks
            queued = set()
            PACE = B("PACE", 3)

            def _enq(g, kt):
                if (g, kt) not in queued:
                    queued.add((g, kt))
                    ready.append((g, kt))

            def _drain(n):
                while n > 0 and ready:
                    g, kt = ready.pop(0)
                    _att_block(g, kt)
                    if kt == gstate[g]["last_kt"]:
                        for h in range(HPC):
                            _att_epilogue(g, h)
                    n -= 1

            # ------------- fused QKV + attention g-pipeline --------------
            if True:
                # DMA_ENGINES serves queues round-robin but is serial overall
                # in the cost model, so everything phase-1-critical goes on
                # the SP queue in exact dependency order: wqh, xsh(g0), wql,
                # xsl(g0), rope q-consts, wk, rope k-consts, wv, xs(g1..3).
                NCH = 4                       # x chunks per group
                CW = NDT // NCH               # d-tiles per chunk
                wts = {}

                def _load_w(nm, h):
                    """per-head half of a w tensor: [P, NDT, P]"""
                    t = proj_sb.tile([P, NDT, P], fp8, tag=f"{nm}{h}",
                                     name=f"{nm}{h}")
                    nc.sync.dma_start(t[:], w_in[nm][h])
                    wts[(nm, h)] = t

                class _XS:
                    """xs[:, us, ...] pair-slicing across 4 chunk tiles."""
                    def __init__(self, chunks):
                        self.chunks = chunks
                    def __getitem__(self, idx):
                        us = idx[1]
                        c, lo = us.start // CW, us.start % CW
                        rest = idx[2:]
                        return self.chunks[c][
                            (slice(None), slice(lo, lo + 2)) + rest]

                def _load_xs(part, src, sl, eng=None):
                    eng = eng or nc.sync
                    chunks = []
                    for c in range(NCH):
                        cs = slice(c * CW, (c + 1) * CW)
                        t = proj_sb.tile([P, CW, QG], fp8,
                                         tag=f"xs{part}{c}", bufs=2,
                                         name=f"xs{part}{c}")
                        eng.dma_start(t[:], src[:, cs, sl])
                        chunks.append(t)
                    return _XS(chunks)

                rope_t = {}

                def _load_rope(*names):
                    for nm in names:
                        src = {"cq": cq, "sq": sq, "ck": ck, "sk": sk}[nm]
                        t = proj_sb.tile([DH, S], bf16, tag=nm, name=nm)
                        nc.sync.dma_start(t[:], src[:])
                        rope_t[nm] = t

                def _load_wv(nm):
                    t = proj_sb.tile([P, NDT, NH], fp8, tag=nm, name=nm)
                    nc.sync.dma_start(t[:], w_in[nm][:])
                    wts[nm] = t

                sl0 = slice(0, QG)
                _load_w("wqh", 0)
                xsh0 = _load_xs("h", xh, sl0)
                _load_w("wql", 0)
                xsl0 = _load_xs("l", xl, sl0)
                for nm in ("wqh", "wql", "wkh", "wkl"):
                    for h in range(HPC):
                        if (nm, h) not in wts:
                            _load_w(nm, h)
                _load_wv("wvh")
                _load_wv("wvl")
                _load_rope("cq", "sq", "ck", "sk")

                for g in range(NQG):
                    sl = slice(g * QG, (g + 1) * QG)
                    if g == 0:
                        xsh, xsl = xsh0, xsl0
                    else:
                        xsh = _load_xs("h", xh, sl)
                        xsl = _load_xs("l", xl, sl)

                    kinds_order = (("q", qt, "cq", "sq"),
                                   ("k", kt_, "ck", "sk"))
                    kinds_g = block_kind[g]
                    act_g = [kt for kt in range(NKT)
                             if kinds_g[kt][0] != SKIP]
                    for kind, dst, cn, sn in kinds_order:
                        for h in range(HPC):
                            wh_t = wts[(f"w{kind}h", h)]
                            wl_t = wts[(f"w{kind}l", h)]
                            ps = ps_proj.tile([P, QG], f32, tag="pq",
                                              name="pq")
                            n = 0
                            for mv, st in ((xsh, wh_t), (xsh, wl_t),
                                           (xsl, wh_t)):
                                for u in range(NDT // 2):
                                    us = slice(2 * u, 2 * u + 2)
                                    nc.tensor.matmul(
                                        ps[:], st[:, us], mv[:, us],
                                        start=(n == 0), stop=(n == 23),
                                        perf_mode=DR)
                                    n += 1
                            # psum (16384*raw) -> bf16 raw on ACT
                            raw = rope_sb.tile([P, QG], bf16, tag="raw",
                                               name="raw")
                            nc.scalar.copy(raw[:], ps[:])
                            # rope (quadrant-interleaved pair layout):
                            # out = raw*c + shuffle(raw)*s, where the
                            # shuffle swaps slots t<->t+16 in each quadrant
                            m = dst[h]
                            sw = rope_sb.tile([P, QG], bf16, tag="sw",
                                              name="sw")
                            nc.vector.stream_shuffle(
                                sw[:], raw[:],
                                [(i + 16) % 32 for i in range(32)])
                            nc.vector.tensor_mul(sw[:], sw[:],
                                                 rope_t[sn][:, sl])
                            nc.vector.tensor_mul(m[:, sl], raw[:],
                                                 rope_t[cn][:, sl])
                            nc.vector.tensor_add(m[:, sl], m[:, sl], sw[:])
                            _drain(PACE)
                        if kind == "q" and act_g:
                            # blocks of g over previously computed k/v
                            # tiles are ready as soon as q(g) is roped
                            _att_begin(g)
                            for kt in act_g:
                                if kt < 4 * g:
                                    _enq(g, kt)
                    wvh_t, wvl_t = wts["wvh"], wts["wvl"]
                    for st_i in range(g * 4, g * 4 + 4):
                        lsl = slice((st_i % 4) * P, (st_i % 4) * P + P)
                        ps = ps_proj.tile([P, QG], f32, tag="pq",
                                          name="pq")
                        n = 0
                        for mv, stw in ((xsh, wvh_t), (xsh, wvl_t),
                                        (xsl, wvh_t)):
                            for u in range(NDT // 2):
                                us = slice(2 * u, 2 * u + 2)
                                nc.tensor.matmul(
                                    ps[:, 0:NH], mv[:, us, lsl],
                                    stw[:, us],
                                    start=(n == 0), stop=(n == 23),
                                    perf_mode=DR)
                                n += 1
                        nc.scalar.copy(vt[:, st_i], ps[:, 0:NH])
                        # the diagonal block kt == st_i only needs v tiles
                        # up to st_i: ready as soon as its v group lands
                        if act_g and st_i in act_g:
                            _enq(g, st_i)
                        _drain(PACE)
                    for kt in act_g:
                        _enq(g, kt)

            # wo loads: issued on the SP queue AFTER all x loads so they
            # never delay the x feed; DMA has slack during attention.
            wo_t = []
            for mg in range(NQG):
                msl = slice(mg * QG, (mg + 1) * QG)
                wh_t = wo_sb.tile([P, NKT, QG], fp8, tag=f"woh{mg}",
                                  name=f"woh{mg}")
                nc.sync.dma_start(wh_t[:], woh[:, :, msl])
                wl_t = wo_sb.tile([P, NKT, QG], fp8, tag=f"wol{mg}",
                                  name=f"wol{mg}")
                nc.sync.dma_start(wl_t[:], wol[:, :, msl])
                wo_t.append((wh_t, wl_t))

            # ------------- tail: remaining attention + out-projection ----
            def _proj(h, mg, csl_):
                width = csl_.stop - csl_.start
                wh_t, wl_t = wo_t[mg]
                psr = ps_proj.tile([P, QG], f32, tag="pq", name="pq")
                n = 0
                for st, mv in ((oh_sb[h], wh_t), (ol_sb[h], wh_t),
                               (oh_sb[h], wl_t)):
                    for u in range(NKT // 2):
                        us = slice(2 * u, 2 * u + 2)
                        nc.tensor.matmul(
                            psr[:, 0:width], st[:, us], mv[:, us, csl_],
                            start=(n == 0), stop=(n == 23),
                            perf_mode=DR)
                        n += 1
                rs = r_sb.tile([P, QG], f32, tag="rs", name="rs")
                nc.scalar.activation(rs[:, 0:width], psr[:, 0:width],
                                     COPY, scale=1.0 / 16384.0)
                msl = slice(mg * QG + csl_.start,
                            mg * QG + csl_.stop)
                nc.sync.dma_start(out[h * P:(h + 1) * P, msl],
                                  rs[:, 0:width])

            proj_jobs = [(h, mg) for h in range(HPC) for mg in range(NQG)]
            while ready:
                _drain(1)
            for i, (h, mg) in enumerate(proj_jobs):
                if i == len(proj_jobs) - 1:
                    # split the last group so the final copy+store tail
                    # is half as long
                    _proj(h, mg, slice(0, QG // 2))
                    _proj(h, mg, slice(QG // 2, QG))
                else:
                    _proj(h, mg, slice(0, QG))

    nc.compile()
    return nc


def _classify_mask(maskT):
    """Per (g, kt) block of maskT [k, q]: (SKIP|NOMASK|CAUSAL|MASKED, off)."""
    kinds = []
    qi = np.arange(QG)
    ki = np.arange(P)
    for g in range(NQG):
        row = []
        for kt in range(NKT):
            blk = maskT[kt * P:(kt + 1) * P, g * QG:(g + 1) * QG]
            if np.all(blk <= -1e5):
                row.append((SKIP, 0))
                continue
            if not blk.any():
                row.append((NOMASK, 0))
                continue
            c = kt - 4 * g
            if 0 <= c <= 3:
                keep = (g * QG + qi[None, :]) >= (kt * P + ki[:, None])
                if (np.all((blk == 0) == keep)
                        and np.all(blk[~keep] <= -1e5)):
                    row.append((CAUSAL, P * c))
                    continue
            off = 0
            while (off + P <= QG - P
                   and np.all(blk[:, off:off + P] <= -1e5)):
                off += P
            row.append((MASKED, off))
        kinds.append(tuple(row))
    return tuple(kinds)


def _get_nc(block_kind):
    key = ("nc", block_kind)
    if key not in _CACHE:
        _CACHE[key] = _build(block_kind)
    return _CACHE[key]


def _prep_inputs(x, freqs_cos, freqs_sin, mask, wq, wk, wv, wo):
    import ml_dtypes
    f = np.float32
    E4 = ml_dtypes.float8_e4m3
    BF = ml_dtypes.bfloat16

    x = np.asarray(x, f).reshape(S, D)
    mask = np.asarray(mask, f).reshape(S, S)
    wq, wk, wv, wo = (np.asarray(w, f) for w in (wq, wk, wv, wo))
    cos = np.asarray(freqs_cos, f)
    sin = np.asarray(freqs_sin, f)

    def split8(a):
        hi = a.astype(E4)
        lo = (a - hi.astype(f)).astype(E4)
        return hi, lo

    def to_pts(a2d, scale):
        """[R, C] f32 -> hi/lo fp8 in [P, R//P, C] layout."""
        a = a2d * scale
        hi, lo = split8(a)
        def lay(t):
            return np.ascontiguousarray(
                t.reshape(R // P, P, -1).transpose(1, 0, 2))
        R = a2d.shape[0]
        return lay(hi), lay(lo)

    xT = np.ascontiguousarray(x.T)                     # [D, S]
    xh_, xl_ = to_pts(xT, 16.0)

    # quadrant-interleaved rope layout for q/k head dims: quadrant q's
    # 32 slots hold even dims 2*(16q+t) for t<16, then their odd partners.
    # The t<->t+16 swap is then a DVE stream_shuffle (intra-quadrant).
    perm = np.empty(DH, np.int64)
    r = np.arange(DH)
    quad, t = r // 32, r % 32
    perm[:] = np.where(t < 16, 32 * quad + 2 * (t % 16),
                       32 * quad + 2 * (t % 16) + 1)
    jidx = 16 * quad + (t % 16)        # freq index per row
    maskT = np.ascontiguousarray(mask.T)

    C2 = cos.T[jidx]                                   # [128, S]
    S2 = np.where((t < 16)[:, None], -sin.T[jidx], sin.T[jidx])
    sc_q = (1.0 / 16384.0) / math.sqrt(DH)
    sc_k = 1.0 / 16384.0

    woT = np.ascontiguousarray(wo.T)                   # [j, m]
    woh_, wol_ = to_pts(woT, 1024.0)

    common = {
        "xh": xh_, "xl": xl_, "maskT": maskT,
        "woh": woh_, "wol": wol_,
        "cq": np.ascontiguousarray(C2 * sc_q).astype(BF),
        "sq": np.ascontiguousarray(S2 * sc_q).astype(BF),
        "ck": np.ascontiguousarray(C2 * sc_k).astype(BF),
        "sk": np.ascontiguousarray(S2 * sc_k).astype(BF),
        # sums moving vector: folds the 2^10 fp8-headroom scale so the
        # reciprocal of the raw PSUM sums is directly the oh/ol scale
        "ones": np.full((P, 1), 1024.0, BF),
        "cmask": np.broadcast_to(
            (np.arange(QG)[None, None, :]
             >= np.arange(P)[:, None, None]), (P, HPC, QG)).astype(BF),
    }
    in_maps = []
    for c in range(N_CORES):
        rows = np.arange(c * NH, (c + 1) * NH)
        # permuted row order for q/k: per head [evens; odds]
        rows_pm = np.concatenate(
            [c * NH + h * DH + perm for h in range(HPC)])
        m = dict(common)
        for kind, w in (("q", wq), ("k", wk), ("v", wv)):
            r = rows_pm if kind in ("q", "k") else rows
            wT = np.ascontiguousarray(w[r].T)          # [D, NH]
            hi, lo = to_pts(wT, 1024.0)                # [P, NDT, NH]
            if kind != "v":
                # per-head-major: [HPC, P, NDT, P]
                hi = np.ascontiguousarray(
                    hi.reshape(P, NDT, HPC, P).transpose(2, 0, 1, 3))
                lo = np.ascontiguousarray(
                    lo.reshape(P, NDT, HPC, P).transpose(2, 0, 1, 3))
            m[f"w{kind}h"] = hi
            m[f"w{kind}l"] = lo
        in_maps.append(m)
    return in_maps


def kernel(x, freqs_cos, freqs_sin, mask, wq, wk, wv, wo, start_pos):
    from concourse.bass_utils import run_bass_kernel_spmd

    in_maps = _prep_inputs(x, freqs_cos, freqs_sin, mask, wq, wk, wv, wo)
    nc = _get_nc(_classify_mask(in_maps[0]["maskT"]))
    res = run_bass_kernel_spmd(nc, in_maps, core_ids=list(range(N_CORES)))
    full = np.concatenate([res.results[c]["out"] for c in range(N_CORES)],
                          axis=0)
    return full.reshape(1, S, D).astype(np.float32)



# revision 24
# speedup vs baseline: 1.1080x; 1.0854x over previous
"""Trainium2 Bass kernel for a 16-head dense attention block (B=1, S=2048, D=2048).

Sharding: 2 heads per core across 8 cores (tensor parallel on heads).
The reference's (deliberate) transpose(2,3)+reshape before the output
projection makes output rows [h*128:(h+1)*128) depend ONLY on head h, so
per-core outputs are disjoint row blocks -> host-side concat, no collectives.

V2: fp8 DoubleRow compensated matmuls + bf16 attention.
  * QKV projections and out-projection run as 3-pass hi/lo-compensated
    fp8e4m3 DoubleRow matmuls (contraction 256/instr at 0.5 cyc/row =
    25% fewer PE cycles than f32r, half the HBM bytes). hi and lo are
    quantized at the SAME power-2 scale so all 3 passes accumulate into
    one PSUM group with no combine ops.
  * Scale ledger: x*16 (hi/lo), w*1024 (hi/lo) -> psum q/k/v = 16384*true.
    q/k: 1/16384 (and q's 1/sqrt(dh)) folded into bf16 rope constants.
    v: kept raw (16384*v, bf16); the factor rides through AV into O and is
    removed by the oh/ol quantization scale (rt = 1/(1024*sums); 16384/1024
    = 16 = fp8 headroom scale for O). wo*1024 -> out copy scales 1/16384.
  * Rope in rotate-half layout (wq/wk rows pre-permuted host-side to
    [evens; odds]) -> pure partition-offset DVE ops on bf16, no swap DMAs.
  * Scores/AV/sums all bf16 (q/k/v/probs); exp on ACT writes bf16 for
    both heads in one op (scores for both heads share one 2-bank PSUM).
  * Causal masking via affine_select (iota predicate qf >= kp) on the exp
    output - no mask DMA, no mask add. Non-causal masks fall back to the
    mask-load + add path per block. Diagonal blocks fully trimmed
    (off = 128*c; bf16 matmul has no >=256 free-size requirement).
  * Per-g sums for both heads bounce via one DRAM round trip to
    redistribute into per-partition layout for the O^T->O epilogue scale.
"""

import math

import numpy as np

S = 2048
D = 2048
H = 16
DH = 128
N_CORES = 8
HPC = H // N_CORES          # heads per core
NH = HPC * DH               # per-core head rows (256)
P = 128
HALF = 64
QG = 512                    # q/s-group width
NQG = S // QG               # 4
NKT = S // P                # 16 k tiles
NDT = D // P                # 16 d tiles

SKIP, NOMASK, CAUSAL, MASKED = 0, 1, 2, 3

_CACHE = {}


def _build(block_kind):
    """block_kind: tuple of NQG tuples of NKT (kind, off) pairs."""
    import os
    import concourse.tile as tile
    from concourse import bacc, mybir

    B = lambda k, d: int(os.environ.get(k, d))
    _os = os
    f32 = mybir.dt.float32
    f32r = mybir.dt.float32r
    bf16 = mybir.dt.bfloat16
    fp8 = mybir.dt.float8e4
    EXP = mybir.ActivationFunctionType.Exp
    COPY = mybir.ActivationFunctionType.Copy
    DR = mybir.MatmulPerfMode.DoubleRow
    MUL = mybir.AluOpType.mult
    SUB = mybir.AluOpType.subtract

    nc = bacc.Bacc("TRN2", target_bir_lowering=False, debug=False,
                   num_devices=N_CORES)

    xh = nc.dram_tensor("xh", [P, NDT, S], fp8, kind="ExternalInput").ap()
    xl = nc.dram_tensor("xl", [P, NDT, S], fp8, kind="ExternalInput").ap()
    w_in = {}
    for kind in ("q", "k", "v"):
        for part in ("h", "l"):
            nm = f"w{kind}{part}"
            shape = [P, NDT, NH] if kind == "v" else [HPC, P, NDT, P]
            w_in[nm] = nc.dram_tensor(nm, shape, fp8,
                                      kind="ExternalInput").ap()
    woh = nc.dram_tensor("woh", [P, NKT, D], fp8, kind="ExternalInput").ap()
    wol = nc.dram_tensor("wol", [P, NKT, D], fp8, kind="ExternalInput").ap()
    maskT = nc.dram_tensor("maskT", [S, S], f32, kind="ExternalInput").ap()
    cq = nc.dram_tensor("cq", [DH, S], bf16, kind="ExternalInput").ap()
    sq = nc.dram_tensor("sq", [DH, S], bf16, kind="ExternalInput").ap()
    ck = nc.dram_tensor("ck", [DH, S], bf16, kind="ExternalInput").ap()
    sk = nc.dram_tensor("sk", [DH, S], bf16, kind="ExternalInput").ap()
    ones = nc.dram_tensor("ones", [P, 1], bf16, kind="ExternalInput").ap()
    cmask = nc.dram_tensor("cmask", [P, HPC, QG], bf16,
                           kind="ExternalInput").ap()
    out = nc.dram_tensor("out", [NH, D], f32, kind="ExternalOutput").ap()

    with tile.TileContext(nc) as tc:
        with tc.tile_pool(name="consts", bufs=1) as consts, \
             tc.tile_pool(name="qkv", bufs=1) as qkv, \
             tc.tile_pool(name="att_p", bufs=1) as att_p, \
             tc.tile_pool(name="wo_sb", bufs=B("BW", 1)) as wo_sb, \
             tc.tile_pool(name="proj_sb", bufs=1) as proj_sb, \
             tc.tile_pool(name="rope_sb", bufs=B("BR", 4)) as rope_sb, \
             tc.tile_pool(name="att_sb", bufs=B("BA", 4)) as att_sb, \
             tc.tile_pool(name="r_sb", bufs=B("BRS", 4)) as r_sb, \
             tc.tile_pool(name="ps_proj", bufs=B("BP", 2), space="PSUM") as ps_proj, \
             tc.tile_pool(name="ps_sc", bufs=B("BS", 3), space="PSUM") as ps_sc, \
             tc.tile_pool(name="ps_o", bufs=B("BO", 1), space="PSUM") as ps_o, \
             tc.tile_pool(name="ps_sum", bufs=B("BSM", 1), space="PSUM") as ps_sum:

            ones_t = consts.tile([P, 1], bf16, tag="ones")
            nc.scalar.dma_start(ones_t[:], ones[:])
            cm_t = consts.tile([P, HPC, QG], bf16, tag="cmask")
            nc.scalar.dma_start(cm_t[:], cmask[:])
            # warm the Exp activation table off the critical path
            warm = consts.tile([1, 1], bf16, tag="warm")
            nc.scalar.activation(warm[:], ones_t[0:1, 0:1], EXP)

            qt = [qkv.tile([P, S], bf16, tag=f"qt{h}", name=f"qt{h}")
                  for h in range(HPC)]
            kt_ = [qkv.tile([P, S], bf16, tag=f"kt{h}", name=f"kt{h}")
                   for h in range(HPC)]
            vt = qkv.tile([P, NKT, NH], bf16, tag="v")   # [k-part, ktile, n]

            oh_sb = [att_p.tile([P, NKT, P], fp8, tag=f"oh{h}",
                                name=f"oh{h}") for h in range(HPC)]
            ol_sb = [att_p.tile([P, NKT, P], fp8, tag=f"ol{h}",
                                name=f"ol{h}") for h in range(HPC)]
            rt = att_p.tile([P, HPC, NKT], f32, tag="rt", name="rt")
            maskT_v = maskT.rearrange("(t p) s -> t p s", p=P)

            # ---- attention block emitters (drained between QKV groups) --
            # probs are the STATIONARY operand of both the AV and the sums
            # matmuls: AV emits O directly in [s, dh] layout (no epilogue
            # PE transposes) and the sums matmul has output free-size 1
            # (~free in the PE cost model vs a full moving-width pass).
            # exp runs once per (g, kt) over both heads' scores ([P,2,nw]).
            # PSUM accumulation start=True zeroes the tile's whole 2KB zero
            # region, so emit exactly ONE start (first matmul into the
            # tile) and ONE stop (last matmul); per-chunk first touches
            # overwrite via pending-zero.
            gstate = {}

            def _att_begin(g):
                kinds = block_kind[g]
                active = [kt for kt in range(NKT) if kinds[kt][0] != SKIP]
                first = active[0]
                offs = {kt: (0 if kt == first else kinds[kt][1])
                        for kt in active}
                gstate[g] = dict(
                    kinds=kinds, active=active, first=first, offs=offs,
                    last_kt=active[-1], po_first=[True] * HPC,
                    pss_first=[True],
                    pss=ps_sum.tile([P, HPC, 4], f32, tag="pss",
                                    name="pss"),
                    po=[ps_o.tile([P, 4, P], f32, tag=f"po{h}",
                                  name=f"po{h}") for h in range(HPC)])

            def _att_block(g, kt):
                # emitted as two half-width (256-col) pieces so the scores
                # psum ring (1-bank tiles, bufs>=2) pipelines sc(i+1) with
                # exp(i) instead of serializing the PE<->ACT chain
                st_ = gstate[g]
                bkind, off = st_["kinds"][kt][0], st_["offs"][kt]
                po, pss = st_["po"], st_["pss"]
                ksl = slice(kt * P, (kt + 1) * P)
                HW_ = QG // 2
                for half in (0, 1):
                    hstart = half * HW_
                    hoff = max(off - hstart, 0)
                    if hoff >= HW_:
                        continue
                    nw = HW_ - hoff
                    qsl = slice(g * QG + hstart + hoff,
                                g * QG + hstart + HW_)
                    osl = slice(hoff, HW_)
                    pt2 = att_sb.tile([P, HPC, HW_], bf16, tag="pt",
                                      name="pt")
                    ps2 = ps_sc.tile([P, HPC, HW_], f32, tag="sc",
                                     name="sc")
                    for h in range(HPC):
                        nc.tensor.matmul(ps2[:, h, osl], kt_[h][:, ksl],
                                         qt[h][:, qsl],
                                         start=True, stop=True)
                    if bkind == MASKED:
                        mt = att_sb.tile([P, HW_], f32, tag="mask",
                                         name="mt")
                        nc.sync.dma_start(mt[:, osl], maskT_v[kt][:, qsl])
                        for h in range(HPC):
                            sm = att_sb.tile([P, HW_], f32, tag="sm",
                                             name="sm")
                            nc.vector.tensor_add(sm[:, osl],
                                                 ps2[:, h, osl],
                                                 mt[:, osl])
                            nc.scalar.activation(pt2[:, h, osl],
                                                 sm[:, osl], EXP)
                    else:
                        nc.scalar.activation(pt2[:, :, osl],
                                             ps2[:, :, osl], EXP)
                        if bkind == CAUSAL:
                            # zero where qf < kp: cm[p, jj] = (jj >= p)
                            # with jj the column offset from the block's
                            # masking origin (off)
                            cs = hstart + hoff - off
                            nc.vector.tensor_mul(pt2[:, :, osl],
                                                 pt2[:, :, osl],
                                                 cm_t[:, :, cs:cs + nw])
                    for h in range(HPC):
                        for cl in range(hoff // P, 2):
                            c = 2 * half + cl
                            csl = slice(cl * P, (cl + 1) * P)
                            sp = (kt == st_["last_kt"] and c == 3)
                            nc.tensor.matmul(
                                po[h][:, c, :], pt2[:, h, csl],
                                vt[:, kt, h * P:(h + 1) * P],
                                start=st_["po_first"][h], stop=sp)
                            st_["po_first"][h] = False
                            nc.tensor.matmul(
                                pss[:, h, c:c + 1], pt2[:, h, csl],
                                ones_t[:],
                                start=st_["pss_first"][0],
                                stop=(sp and h == HPC - 1))
                            st_["pss_first"][0] = False

            def _att_epilogue(g, h):
                # sums land per-partition (p <-> q = jt*128+p): reciprocal
                # and oh/ol quantization read PSUM directly.
                st_ = gstate[g]
                g4 = slice(g * 4, g * 4 + 4)
                nc.vector.reciprocal(rt[:, h, g4], st_["pss"][:, h, 0:4])
                for c in range(4):
                    jt = g * 4 + c
                    rsc = rt[:, h, jt:jt + 1]
                    nc.vector.tensor_scalar_mul(oh_sb[h][:, jt],
                                                st_["po"][h][:, c, :], rsc)
                    nc.vector.scalar_tensor_tensor(
                        ol_sb[h][:, jt], st_["po"][h][:, c, :], rsc,
                        oh_sb[h][:, jt], op0=MUL, op1=SUB)

            ready = []          # FIFO of (g, kt) attention blocks
            queued = set()
            PACE = B("PACE", 3)

            def _enq(g, kt):
                if (g, kt) not in queued:
                    queued.add((g, kt))
                    ready.append((g, kt))

            def _drain(n):
                while n > 0 and ready:
                    g, kt = ready.pop(0)
                    _att_block(g, kt)
                    if kt == gstate[g]["last_kt"]:
                        for h in range(HPC):
                            _att_epilogue(g, h)
                    n -= 1

            # ------------- fused QKV + attention g-pipeline --------------
            if True:
                # DMA_ENGINES serves queues round-robin but is serial overall
                # in the cost model, so everything phase-1-critical goes on
                # the SP queue in exact dependency order: wqh, xsh(g0), wql,
                # xsl(g0), rope q-consts, wk, rope k-consts, wv, xs(g1..3).
                NCH = 4                       # x chunks per group
                CW = NDT // NCH               # d-tiles per chunk
                wts = {}

                def _load_w(nm, h):
                    """per-head half of a w tensor: [P, NDT, P]"""
                    t = proj_sb.tile([P, NDT, P], fp8, tag=f"{nm}{h}",
                                     name=f"{nm}{h}")
                    nc.sync.dma_start(t[:], w_in[nm][h])
                    wts[(nm, h)] = t

                class _XS:
                    """xs[:, us, ...] pair-slicing across 4 chunk tiles."""
                    def __init__(self, chunks):
                        self.chunks = chunks
                    def __getitem__(self, idx):
                        us = idx[1]
                        c, lo = us.start // CW, us.start % CW
                        rest = idx[2:]
                        return self.chunks[c][
                            (slice(None), slice(lo, lo + 2)) + rest]

                def _load_xs(part, src, sl, eng=None):
                    eng = eng or nc.sync
                    chunks = []
                    for c in range(NCH):
                        cs = slice(c * CW, (c + 1) * CW)
                        t = proj_sb.tile([P, CW, QG], fp8,
                                         tag=f"xs{part}{c}", bufs=2,
                                         name=f"xs{part}{c}")
                        eng.dma_start(t[:], src[:, cs, sl])
                        chunks.append(t)
                    return _XS(chunks)

                rope_t = {}

                def _load_rope(*names):
                    for nm in names:
                        src = {"cq": cq, "sq": sq, "ck": ck, "sk": sk}[nm]
                        t = proj_sb.tile([DH, S], bf16, tag=nm, name=nm)
                        nc.sync.dma_start(t[:], src[:])
                        rope_t[nm] = t

                def _load_wv(nm):
                    t = proj_sb.tile([P, NDT, NH], fp8, tag=nm, name=nm)
                    nc.sync.dma_start(t[:], w_in[nm][:])
                    wts[nm] = t

                sl0 = slice(0, QG)
                _load_w("wqh", 0)
                xsh0 = _load_xs("h", xh, sl0)
                _load_w("wql", 0)
                xsl0 = _load_xs("l", xl, sl0)
                for nm in ("wqh", "wql", "wkh", "wkl"):
                    for h in range(HPC):
                        if (nm, h) not in wts:
                            _load_w(nm, h)
                _load_wv("wvh")
                _load_wv("wvl")
                _load_rope("cq", "sq", "ck", "sk")

                for g in range(NQG):
                    sl = slice(g * QG, (g + 1) * QG)
                    if g == 0:
                        xsh, xsl = xsh0, xsl0
                    else:
                        xsh = _load_xs("h", xh, sl)
                        xsl = _load_xs("l", xl, sl)

                    kinds_order = (("q", qt, "cq", "sq"),
                                   ("k", kt_, "ck", "sk"))
                    kinds_g = block_kind[g]
                    act_g = [kt for kt in range(NKT)
                             if kinds_g[kt][0] != SKIP]
                    for kind, dst, cn, sn in kinds_order:
                        for h in range(HPC):
                            wh_t = wts[(f"w{kind}h", h)]
                            wl_t = wts[(f"w{kind}l", h)]
                            ps = ps_proj.tile([P, QG], f32, tag="pq",
                                              name="pq")
                            n = 0
                            for mv, st in ((xsh, wh_t), (xsh, wl_t),
                                           (xsl, wh_t)):
                                for u in range(NDT // 2):
                                    us = slice(2 * u, 2 * u + 2)
                                    nc.tensor.matmul(
                                        ps[:], st[:, us], mv[:, us],
                                        start=(n == 0), stop=(n == 23),
                                        perf_mode=DR)
                                    n += 1
                            # psum (16384*raw) -> bf16 raw on ACT
                            raw = rope_sb.tile([P, QG], bf16, tag="raw",
                                               name="raw")
                            nc.scalar.copy(raw[:], ps[:])
                            # rope (quadrant-interleaved pair layout):
                            # out = raw*c + shuffle(raw)*s, where the
                            # shuffle swaps slots t<->t+16 in each quadrant
                            m = dst[h]
                            sw = rope_sb.tile([P, QG], bf16, tag="sw",
                                              name="sw")
                            nc.vector.stream_shuffle(
                                sw[:], raw[:],
                                [(i + 16) % 32 for i in range(32)])
                            nc.vector.tensor_mul(sw[:], sw[:],
                                                 rope_t[sn][:, sl])
                            nc.vector.tensor_mul(m[:, sl], raw[:],
                                                 rope_t[cn][:, sl])
                            nc.vector.tensor_add(m[:, sl], m[:, sl], sw[:])
                            _drain(PACE)
                        if kind == "q" and act_g:
                            # blocks of g over previously computed k/v
                            # tiles are ready as soon as q(g) is roped
                            _att_begin(g)
                            for kt in act_g:
                                if kt < 4 * g:
                                    _enq(g, kt)
                    wvh_t, wvl_t = wts["wvh"], wts["wvl"]
                    for st_i in range(g * 4, g * 4 + 4):
                        lsl = slice((st_i % 4) * P, (st_i % 4) * P + P)
                        ps = ps_proj.tile([P, QG], f32, tag="pq",
                                          name="pq")
                        n = 0
                        for mv, stw in ((xsh, wvh_t), (xsh, wvl_t),
                                        (xsl, wvh_t)):
                            for u in range(NDT // 2):
                                us = slice(2 * u, 2 * u + 2)
                                nc.tensor.matmul(
                                    ps[:, 0:NH], mv[:, us, lsl],
                                    stw[:, us],
                                    start=(n == 0), stop=(n == 23),
                                    perf_mode=DR)
                                n += 1
                        nc.scalar.copy(vt[:, st_i], ps[:, 0:NH])
                        # the diagonal block kt == st_i only needs v tiles
                        # up to st_i: ready as soon as its v group lands
                        if act_g and st_i in act_g:
                            _enq(g, st_i)
                        _drain(PACE)
                    for kt in act_g:
                        _enq(g, kt)

            # wo loads: issued on the SP queue AFTER all x loads so they
            # never delay the x feed; DMA has slack during attention.
            wo_t = []
            for mg in range(NQG):
                msl = slice(mg * QG, (mg + 1) * QG)
                wh_t = wo_sb.tile([P, NKT, QG], fp8, tag=f"woh{mg}",
                                  name=f"woh{mg}")
                nc.sync.dma_start(wh_t[:], woh[:, :, msl])
                wl_t = wo_sb.tile([P, NKT, QG], fp8, tag=f"wol{mg}",
                                  name=f"wol{mg}")
                nc.sync.dma_start(wl_t[:], wol[:, :, msl])
                wo_t.append((wh_t, wl_t))

            # ------------- tail: remaining attention + out-projection ----
            def _proj(h, mg, csl_):
                width = csl_.stop - csl_.start
                wh_t, wl_t = wo_t[mg]
                psr = ps_proj.tile([P, QG], f32, tag="pq", name="pq")
                n = 0
                for st, mv in ((oh_sb[h], wh_t), (ol_sb[h], wh_t),
                               (oh_sb[h], wl_t)):
                    for u in range(NKT // 2):
                        us = slice(2 * u, 2 * u + 2)
                        nc.tensor.matmul(
                            psr[:, 0:width], st[:, us], mv[:, us, csl_],
                            start=(n == 0), stop=(n == 23),
                            perf_mode=DR)
                        n += 1
                rs = r_sb.tile([P, QG], f32, tag="rs", name="rs")
                nc.scalar.activation(rs[:, 0:width], psr[:, 0:width],
                                     COPY, scale=1.0 / 16384.0)
                msl = slice(mg * QG + csl_.start,
                            mg * QG + csl_.stop)
                nc.sync.dma_start(out[h * P:(h + 1) * P, msl],
                                  rs[:, 0:width])

            proj_jobs = [(h, mg) for h in range(HPC) for mg in range(NQG)]
            while ready:
                _drain(1)
            for i, (h, mg) in enumerate(proj_jobs):
                if i == len(proj_jobs) - 1:
                    # split the last group so the final copy+store tail
                    # is half as long
                    _proj(h, mg, slice(0, QG // 2))
                    _proj(h, mg, slice(QG // 2, QG))
                else:
                    _proj(h, mg, slice(0, QG))

    nc.compile()
    return nc


def _classify_mask(maskT):
    """Per (g, kt) block of maskT [k, q]: (SKIP|NOMASK|CAUSAL|MASKED, off)."""
    kinds = []
    qi = np.arange(QG)
    ki = np.arange(P)
    for g in range(NQG):
        row = []
        for kt in range(NKT):
            blk = maskT[kt * P:(kt + 1) * P, g * QG:(g + 1) * QG]
            if np.all(blk <= -1e5):
                row.append((SKIP, 0))
                continue
            if not blk.any():
                row.append((NOMASK, 0))
                continue
            c = kt - 4 * g
            if 0 <= c <= 3:
                keep = (g * QG + qi[None, :]) >= (kt * P + ki[:, None])
                if (np.all((blk == 0) == keep)
                        and np.all(blk[~keep] <= -1e5)):
                    row.append((CAUSAL, P * c))
                    continue
            off = 0
            while (off + P <= QG - P
                   and np.all(blk[:, off:off + P] <= -1e5)):
                off += P
            row.append((MASKED, off))
        kinds.append(tuple(row))
    return tuple(kinds)


def _get_nc(block_kind):
    key = ("nc", block_kind)
    if key not in _CACHE:
        _CACHE[key] = _build(block_kind)
    return _CACHE[key]


def _prep_inputs(x, freqs_cos, freqs_sin, mask, wq, wk, wv, wo):
    import ml_dtypes
    f = np.float32
    E4 = ml_dtypes.float8_e4m3
    BF = ml_dtypes.bfloat16

    x = np.asarray(x, f).reshape(S, D)
    mask = np.asarray(mask, f).reshape(S, S)
    wq, wk, wv, wo = (np.asarray(w, f) for w in (wq, wk, wv, wo))
    cos = np.asarray(freqs_cos, f)
    sin = np.asarray(freqs_sin, f)

    def split8(a):
        hi = a.astype(E4)
        lo = (a - hi.astype(f)).astype(E4)
        return hi, lo

    def to_pts(a2d, scale):
        """[R, C] f32 -> hi/lo fp8 in [P, R//P, C] layout."""
        a = a2d * scale
        hi, lo = split8(a)
        def lay(t):
            return np.ascontiguousarray(
                t.reshape(R // P, P, -1).transpose(1, 0, 2))
        R = a2d.shape[0]
        return lay(hi), lay(lo)

    xT = np.ascontiguousarray(x.T)                     # [D, S]
    xh_, xl_ = to_pts(xT, 16.0)

    # quadrant-interleaved rope layout for q/k head dims: quadrant q's
    # 32 slots hold even dims 2*(16q+t) for t<16, then their odd partners.
    # The t<->t+16 swap is then a DVE stream_shuffle (intra-quadrant).
    perm = np.empty(DH, np.int64)
    r = np.arange(DH)
    quad, t = r // 32, r % 32
    perm[:] = np.where(t < 16, 32 * quad + 2 * (t % 16),
                       32 * quad + 2 * (t % 16) + 1)
    jidx = 16 * quad + (t % 16)        # freq index per row
    maskT = np.ascontiguousarray(mask.T)

    C2 = cos.T[jidx]                                   # [128, S]
    S2 = np.where((t < 16)[:, None], -sin.T[jidx], sin.T[jidx])
    sc_q = (1.0 / 16384.0) / math.sqrt(DH)
    sc_k = 1.0 / 16384.0

    woT = np.ascontiguousarray(wo.T)                   # [j, m]
    woh_, wol_ = to_pts(woT, 1024.0)

    common = {
        "xh": xh_, "xl": xl_, "maskT": maskT,
        "woh": woh_, "wol": wol_,
        "cq": np.ascontiguousarray(C2 * sc_q).astype(BF),
        "sq": np.ascontiguousarray(S2 * sc_q).astype(BF),
        "ck": np.ascontiguousarray(C2 * sc_k).astype(BF),
        "sk": np.ascontiguousarray(S2 * sc_k).astype(BF),
        # sums moving vector: folds the 2^10 fp8-headroom scale so the
        # reciprocal of the raw PSUM sums is directly the oh/ol scale
        "ones": np.full((P, 1), 1024.0, BF),
        "cmask": np.broadcast_to(
            (np.arange(QG)[None, None, :]
             >= np.arange(P)[:, None, None]), (P, HPC, QG)).astype(BF),
    }
    in_maps = []
    for c in range(N_CORES):
        rows = np.arange(c * NH, (c + 1) * NH)
        # permuted row order for q/k: per head [evens; odds]
        rows_pm = np.concatenate(
            [c * NH + h * DH + perm for h in range(HPC)])
        m = dict(common)
        for kind, w in (("q", wq), ("k", wk), ("v", wv)):
            r = rows_pm if kind in ("q", "k") else rows
            wT = np.ascontiguousarray(w[r].T)          # [D, NH]
            hi, lo = to_pts(wT, 1024.0)                # [P, NDT, NH]
            if kind != "v":
                # per-head-major: [HPC, P, NDT, P]
                hi = np.ascontiguousarray(
                    hi.reshape(P, NDT, HPC, P).transpose(2, 0, 1, 3))
                lo = np.ascontiguousarray(
                    lo.reshape(P, NDT, HPC, P).transpose(2, 0, 1, 3))
            m[f"w{kind}h"] = hi
            m[f"w{kind}l"] = lo
        in_maps.append(m)
    return in_maps


def kernel(x, freqs_cos, freqs_sin, mask, wq, wk, wv, wo, start_pos):
    from concourse.bass_utils import run_bass_kernel_spmd

    in_maps = _prep_inputs(x, freqs_cos, freqs_sin, mask, wq, wk, wv, wo)
    nc = _get_nc(_classify_mask(in_maps[0]["maskT"]))
    res = run_bass_kernel_spmd(nc, in_maps, core_ids=list(range(N_CORES)))
    full = np.concatenate([res.results[c]["out"] for c in range(N_CORES)],
                          axis=0)
    return full.reshape(1, S, D).astype(np.float32)



# revision 29
# speedup vs baseline: 1.1266x; 1.0169x over previous
"""Trainium2 Bass kernel for a 16-head dense attention block (B=1, S=2048, D=2048).

Sharding: 2 heads per core across 8 cores (tensor parallel on heads).
The reference's (deliberate) transpose(2,3)+reshape before the output
projection makes output rows [h*128:(h+1)*128) depend ONLY on head h, so
per-core outputs are disjoint row blocks -> host-side concat, no collectives.

V2: fp8 DoubleRow compensated matmuls + bf16 attention.
  * QKV projections and out-projection run as 3-pass hi/lo-compensated
    fp8e4m3 DoubleRow matmuls (contraction 256/instr at 0.5 cyc/row =
    25% fewer PE cycles than f32r, half the HBM bytes). hi and lo are
    quantized at the SAME power-2 scale so all 3 passes accumulate into
    one PSUM group with no combine ops.
  * Scale ledger: x*16 (hi/lo), w*1024 (hi/lo) -> psum q/k/v = 16384*true.
    q/k: 1/16384 (and q's 1/sqrt(dh)) folded into bf16 rope constants.
    v: kept raw (16384*v, bf16); the factor rides through AV into O and is
    removed by the oh/ol quantization scale (rt = 1/(1024*sums); 16384/1024
    = 16 = fp8 headroom scale for O). wo*1024 -> out copy scales 1/16384.
  * Rope in rotate-half layout (wq/wk rows pre-permuted host-side to
    [evens; odds]) -> pure partition-offset DVE ops on bf16, no swap DMAs.
  * Scores/AV/sums all bf16 (q/k/v/probs); exp on ACT writes bf16 for
    both heads in one op (scores for both heads share one 2-bank PSUM).
  * Causal masking via affine_select (iota predicate qf >= kp) on the exp
    output - no mask DMA, no mask add. Non-causal masks fall back to the
    mask-load + add path per block. Diagonal blocks fully trimmed
    (off = 128*c; bf16 matmul has no >=256 free-size requirement).
  * Per-g sums for both heads bounce via one DRAM round trip to
    redistribute into per-partition layout for the O^T->O epilogue scale.
"""

import math

import numpy as np

S = 2048
D = 2048
H = 16
DH = 128
N_CORES = 8
HPC = H // N_CORES          # heads per core
NH = HPC * DH               # per-core head rows (256)
P = 128
HALF = 64
QG = 512                    # q/s-group width
NQG = S // QG               # 4
NKT = S // P                # 16 k tiles
NDT = D // P                # 16 d tiles

SKIP, NOMASK, CAUSAL, MASKED = 0, 1, 2, 3

_CACHE = {}


def _build(block_kind):
    """block_kind: tuple of NQG tuples of NKT (kind, off) pairs."""
    import os
    import concourse.tile as tile
    from concourse import bacc, mybir

    B = lambda k, d: int(os.environ.get(k, d))
    _os = os
    f32 = mybir.dt.float32
    f32r = mybir.dt.float32r
    bf16 = mybir.dt.bfloat16
    fp8 = mybir.dt.float8e4
    EXP = mybir.ActivationFunctionType.Exp
    COPY = mybir.ActivationFunctionType.Copy
    DR = mybir.MatmulPerfMode.DoubleRow
    MUL = mybir.AluOpType.mult
    SUB = mybir.AluOpType.subtract

    nc = bacc.Bacc("TRN2", target_bir_lowering=False, debug=False,
                   num_devices=N_CORES)

    xh = nc.dram_tensor("xh", [P, NDT, S], fp8, kind="ExternalInput").ap()
    xl = nc.dram_tensor("xl", [P, NDT, S], fp8, kind="ExternalInput").ap()
    w_in = {}
    for kind in ("q", "k", "v"):
        for part in ("h", "l"):
            nm = f"w{kind}{part}"
            shape = [P, NDT, NH] if kind == "v" else [HPC, P, NDT, P]
            w_in[nm] = nc.dram_tensor(nm, shape, fp8,
                                      kind="ExternalInput").ap()
    woh = nc.dram_tensor("woh", [P, NKT, D], fp8, kind="ExternalInput").ap()
    wol = nc.dram_tensor("wol", [P, NKT, D], fp8, kind="ExternalInput").ap()
    maskT = nc.dram_tensor("maskT", [S, S], f32, kind="ExternalInput").ap()
    cq = nc.dram_tensor("cq", [DH, S], bf16, kind="ExternalInput").ap()
    sq = nc.dram_tensor("sq", [DH, S], bf16, kind="ExternalInput").ap()
    ck = nc.dram_tensor("ck", [DH, S], bf16, kind="ExternalInput").ap()
    sk = nc.dram_tensor("sk", [DH, S], bf16, kind="ExternalInput").ap()
    ones = nc.dram_tensor("ones", [P, 1], bf16, kind="ExternalInput").ap()
    cmask = nc.dram_tensor("cmask", [P, HPC, QG], bf16,
                           kind="ExternalInput").ap()
    out = nc.dram_tensor("out", [NH, D], f32, kind="ExternalOutput").ap()

    with tile.TileContext(nc) as tc:
        with tc.tile_pool(name="consts", bufs=1) as consts, \
             tc.tile_pool(name="qkv", bufs=1) as qkv, \
             tc.tile_pool(name="att_p", bufs=1) as att_p, \
             tc.tile_pool(name="wo_sb", bufs=B("BW", 1)) as wo_sb, \
             tc.tile_pool(name="proj_sb", bufs=1) as proj_sb, \
             tc.tile_pool(name="rope_sb", bufs=B("BR", 4)) as rope_sb, \
             tc.tile_pool(name="att_sb", bufs=B("BA", 4)) as att_sb, \
             tc.tile_pool(name="r_sb", bufs=B("BRS", 4)) as r_sb, \
             tc.tile_pool(name="ps_proj", bufs=B("BP", 2), space="PSUM") as ps_proj, \
             tc.tile_pool(name="ps_sc", bufs=B("BS", 3), space="PSUM") as ps_sc, \
             tc.tile_pool(name="ps_o", bufs=B("BO", 1), space="PSUM") as ps_o, \
             tc.tile_pool(name="ps_sum", bufs=B("BSM", 1), space="PSUM") as ps_sum:

            ones_t = consts.tile([P, 1], bf16, tag="ones")
            nc.scalar.dma_start(ones_t[:], ones[:])
            cm_t = consts.tile([P, HPC, QG], bf16, tag="cmask")
            nc.scalar.dma_start(cm_t[:], cmask[:])
            # warm the Exp activation table off the critical path
            warm = consts.tile([1, 1], bf16, tag="warm")
            nc.scalar.activation(warm[:], ones_t[0:1, 0:1], EXP)

            qt = [qkv.tile([P, S], bf16, tag=f"qt{h}", name=f"qt{h}")
                  for h in range(HPC)]
            kt_ = [qkv.tile([P, S], bf16, tag=f"kt{h}", name=f"kt{h}")
                   for h in range(HPC)]
            vt = qkv.tile([P, NKT, NH], bf16, tag="v")   # [k-part, ktile, n]

            oh_sb = [att_p.tile([P, NKT, P], fp8, tag=f"oh{h}",
                                name=f"oh{h}") for h in range(HPC)]
            ol_sb = [att_p.tile([P, NKT, P], fp8, tag=f"ol{h}",
                                name=f"ol{h}") for h in range(HPC)]
            rt = att_p.tile([P, HPC, NKT], f32, tag="rt", name="rt")
            maskT_v = maskT.rearrange("(t p) s -> t p s", p=P)

            # ---- attention block emitters (drained between QKV groups) --
            # probs are the STATIONARY operand of both the AV and the sums
            # matmuls: AV emits O directly in [s, dh] layout (no epilogue
            # PE transposes) and the sums matmul has output free-size 1
            # (~free in the PE cost model vs a full moving-width pass).
            # exp runs once per (g, kt) over both heads' scores ([P,2,nw]).
            # PSUM accumulation start=True zeroes the tile's whole 2KB zero
            # region, so emit exactly ONE start (first matmul into the
            # tile) and ONE stop (last matmul); per-chunk first touches
            # overwrite via pending-zero.
            gstate = {}

            def _att_begin(g):
                kinds = block_kind[g]
                active = [kt for kt in range(NKT) if kinds[kt][0] != SKIP]
                first = active[0]
                offs = {kt: (0 if kt == first else kinds[kt][1])
                        for kt in active}
                gstate[g] = dict(
                    kinds=kinds, active=active, first=first, offs=offs,
                    last_kt=active[-1], po_first=[True] * HPC,
                    pss_first=[True],
                    last_for=[max((kt for kt in active
                                   if offs[kt] <= c * P), default=None)
                              for c in range(4)],
                    pss=ps_sum.tile([P, HPC, 4], f32, tag="pss",
                                    name="pss"),
                    po=[ps_o.tile([P, 4, P], f32, tag=f"po{h}",
                                  name=f"po{h}") for h in range(HPC)])

            def _att_block(g, kt):
                # emitted as two half-width (256-col) pieces so the scores
                # psum ring (1-bank tiles, bufs>=2) pipelines sc(i+1) with
                # exp(i) instead of serializing the PE<->ACT chain
                st_ = gstate[g]
                bkind, off = st_["kinds"][kt][0], st_["offs"][kt]
                po, pss = st_["po"], st_["pss"]
                ksl = slice(kt * P, (kt + 1) * P)
                HW_ = QG // 2
                for half in (0, 1):
                    hstart = half * HW_
                    hoff = max(off - hstart, 0)
                    if hoff >= HW_:
                        continue
                    nw = HW_ - hoff
                    qsl = slice(g * QG + hstart + hoff,
                                g * QG + hstart + HW_)
                    osl = slice(hoff, HW_)
                    pt2 = att_sb.tile([P, HPC, HW_], bf16, tag="pt",
                                      name="pt")
                    ps2 = ps_sc.tile([P, HPC, HW_], f32, tag="sc",
                                     name="sc")
                    for h in range(HPC):
                        nc.tensor.matmul(ps2[:, h, osl], kt_[h][:, ksl],
                                         qt[h][:, qsl],
                                         start=True, stop=True)
                    if bkind == MASKED:
                        mt = att_sb.tile([P, HW_], f32, tag="mask",
                                         name="mt")
                        nc.sync.dma_start(mt[:, osl], maskT_v[kt][:, qsl])
                        for h in range(HPC):
                            sm = att_sb.tile([P, HW_], f32, tag="sm",
                                             name="sm")
                            nc.vector.tensor_add(sm[:, osl],
                                                 ps2[:, h, osl],
                                                 mt[:, osl])
                            nc.scalar.activation(pt2[:, h, osl],
                                                 sm[:, osl], EXP)
                    else:
                        nc.scalar.activation(pt2[:, :, osl],
                                             ps2[:, :, osl], EXP)
                        if bkind == CAUSAL:
                            # zero where qf < kp: cm[p, jj] = (jj >= p)
                            # with jj the column offset from the block's
                            # masking origin (off)
                            cs = hstart + hoff - off
                            nc.vector.tensor_mul(pt2[:, :, osl],
                                                 pt2[:, :, osl],
                                                 cm_t[:, :, cs:cs + nw])
                    for h in range(HPC):
                        for cl in range(hoff // P, 2):
                            c = 2 * half + cl
                            csl = slice(cl * P, (cl + 1) * P)
                            sp = (kt == st_["last_kt"] and c == 3)
                            nc.tensor.matmul(
                                po[h][:, c, :], pt2[:, h, csl],
                                vt[:, kt, h * P:(h + 1) * P],
                                start=st_["po_first"][h], stop=sp)
                            st_["po_first"][h] = False
                            nc.tensor.matmul(
                                pss[:, h, c:c + 1], pt2[:, h, csl],
                                ones_t[:],
                                start=st_["pss_first"][0],
                                stop=(sp and h == HPC - 1))
                            st_["pss_first"][0] = False
                    # chunk epilogues: a chunk is complete once its last
                    # contributing k block lands, so quantize it right
                    # away instead of serializing a big DVE chain at the
                    # end of the g (reads don't need the group stop)
                    for cl in range(hoff // P, 2):
                        c = 2 * half + cl
                        if kt == st_["last_for"][c]:
                            _att_chunk_epi(g, c)

            def _att_chunk_epi(g, c):
                # sums land per-partition (p <-> q = jt*128+p): reciprocal
                # and oh/ol quantization read PSUM directly.
                st_ = gstate[g]
                jt = g * 4 + c
                for h in range(HPC):
                    rsc = rt[:, h, jt:jt + 1]
                    nc.vector.reciprocal(rsc, st_["pss"][:, h, c:c + 1])
                    nc.vector.tensor_scalar_mul(oh_sb[h][:, jt],
                                                st_["po"][h][:, c, :], rsc)
                    nc.vector.scalar_tensor_tensor(
                        ol_sb[h][:, jt], st_["po"][h][:, c, :], rsc,
                        oh_sb[h][:, jt], op0=MUL, op1=SUB)

            ready = []          # FIFO of (g, kt) attention blocks
            queued = set()
            PACE = B("PACE", 3)

            def _enq(g, kt):
                if (g, kt) not in queued:
                    queued.add((g, kt))
                    ready.append((g, kt))

            def _drain(n):
                while n > 0 and ready:
                    g, kt = ready.pop(0)
                    _att_block(g, kt)
                    n -= 1

            # ------------- fused QKV + attention g-pipeline --------------
            if True:
                # DMA_ENGINES serves queues round-robin but is serial overall
                # in the cost model, so everything phase-1-critical goes on
                # the SP queue in exact dependency order: wqh, xsh(g0), wql,
                # xsl(g0), rope q-consts, wk, rope k-consts, wv, xs(g1..3).
                NCH = 4                       # x chunks per group
                CW = NDT // NCH               # d-tiles per chunk
                wts = {}

                def _load_w(nm, h):
                    """per-head half of a w tensor: [P, NDT, P]"""
                    t = proj_sb.tile([P, NDT, P], fp8, tag=f"{nm}{h}",
                                     name=f"{nm}{h}")
                    nc.sync.dma_start(t[:], w_in[nm][h])
                    wts[(nm, h)] = t

                class _XS:
                    """xs[:, us, ...] pair-slicing across 4 chunk tiles."""
                    def __init__(self, chunks):
                        self.chunks = chunks
                    def __getitem__(self, idx):
                        us = idx[1]
                        c, lo = us.start // CW, us.start % CW
                        rest = idx[2:]
                        return self.chunks[c][
                            (slice(None), slice(lo, lo + 2)) + rest]

                def _load_xs(part, src, sl, eng=None):
                    eng = eng or nc.sync
                    chunks = []
                    for c in range(NCH):
                        cs = slice(c * CW, (c + 1) * CW)
                        t = proj_sb.tile([P, CW, QG], fp8,
                                         tag=f"xs{part}{c}", bufs=2,
                                         name=f"xs{part}{c}")
                        eng.dma_start(t[:], src[:, cs, sl])
                        chunks.append(t)
                    return _XS(chunks)

                rope_t = {}

                def _load_rope(*names):
                    for nm in names:
                        src = {"cq": cq, "sq": sq, "ck": ck, "sk": sk}[nm]
                        t = proj_sb.tile([DH, S], bf16, tag=nm, name=nm)
                        nc.sync.dma_start(t[:], src[:])
                        rope_t[nm] = t

                def _load_wv(nm):
                    t = proj_sb.tile([P, NDT, NH], fp8, tag=nm, name=nm)
                    nc.sync.dma_start(t[:], w_in[nm][:])
                    wts[nm] = t

                sl0 = slice(0, QG)
                _load_w("wqh", 0)
                xsh0 = _load_xs("h", xh, sl0)
                _load_w("wql", 0)
                xsl0 = _load_xs("l", xl, sl0)
                for nm in ("wqh", "wql", "wkh", "wkl"):
                    for h in range(HPC):
                        if (nm, h) not in wts:
                            _load_w(nm, h)
                _load_wv("wvh")
                _load_wv("wvl")
                _load_rope("cq", "sq", "ck", "sk")

                for g in range(NQG):
                    sl = slice(g * QG, (g + 1) * QG)
                    if g == 0:
                        xsh, xsl = xsh0, xsl0
                    else:
                        xsh = _load_xs("h", xh, sl)
                        xsl = _load_xs("l", xl, sl)

                    kinds_order = (("q", qt, "cq", "sq"),
                                   ("k", kt_, "ck", "sk"))
                    kinds_g = block_kind[g]
                    act_g = [kt for kt in range(NKT)
                             if kinds_g[kt][0] != SKIP]
                    for kind, dst, cn, sn in kinds_order:
                        for h in range(HPC):
                            wh_t = wts[(f"w{kind}h", h)]
                            wl_t = wts[(f"w{kind}l", h)]
                            ps = ps_proj.tile([P, QG], f32, tag="pq",
                                              name="pq")
                            n = 0
                            for mv, st in ((xsh, wh_t), (xsh, wl_t),
                                           (xsl, wh_t)):
                                for u in range(NDT // 2):
                                    us = slice(2 * u, 2 * u + 2)
                                    nc.tensor.matmul(
                                        ps[:], st[:, us], mv[:, us],
                                        start=(n == 0), stop=(n == 23),
                                        perf_mode=DR)
                                    n += 1
                            # psum (16384*raw) -> bf16 raw on ACT
                            raw = rope_sb.tile([P, QG], bf16, tag="raw",
                                               name="raw")
                            nc.scalar.copy(raw[:], ps[:])
                            # rope (quadrant-interleaved pair layout):
                            # out = raw*c + shuffle(raw)*s, where the
                            # shuffle swaps slots t<->t+16 in each quadrant
                            m = dst[h]
                            sw = rope_sb.tile([P, QG], bf16, tag="sw",
                                              name="sw")
                            nc.vector.stream_shuffle(
                                sw[:], raw[:],
                                [(i + 16) % 32 for i in range(32)])
                            nc.vector.tensor_mul(sw[:], sw[:],
                                                 rope_t[sn][:, sl])
                            nc.vector.tensor_mul(m[:, sl], raw[:],
                                                 rope_t[cn][:, sl])
                            nc.vector.tensor_add(m[:, sl], m[:, sl], sw[:])
                            _drain(PACE)
                        if kind == "q" and act_g:
                            # blocks of g over previously computed k/v
                            # tiles are ready as soon as q(g) is roped
                            _att_begin(g)
                            for kt in act_g:
                                if kt < 4 * g:
                                    _enq(g, kt)
                    wvh_t, wvl_t = wts["wvh"], wts["wvl"]
                    for st_i in range(g * 4, g * 4 + 4):
                        lsl = slice((st_i % 4) * P, (st_i % 4) * P + P)
                        ps = ps_proj.tile([P, QG], f32, tag="pq",
                                          name="pq")
                        n = 0
                        for mv, stw in ((xsh, wvh_t), (xsh, wvl_t),
                                        (xsl, wvh_t)):
                            for u in range(NDT // 2):
                                us = slice(2 * u, 2 * u + 2)
                                nc.tensor.matmul(
                                    ps[:, 0:NH], mv[:, us, lsl],
                                    stw[:, us],
                                    start=(n == 0), stop=(n == 23),
                                    perf_mode=DR)
                                n += 1
                        nc.scalar.copy(vt[:, st_i], ps[:, 0:NH])
                        # the diagonal block kt == st_i only needs v tiles
                        # up to st_i: ready as soon as its v group lands
                        if act_g and st_i in act_g:
                            _enq(g, st_i)
                        _drain(PACE)
                    for kt in act_g:
                        _enq(g, kt)

            # wo loads: issued on the SP queue AFTER all x loads so they
            # never delay the x feed; DMA has slack during attention.
            wo_t = []
            for mg in range(NQG):
                msl = slice(mg * QG, (mg + 1) * QG)
                wh_t = wo_sb.tile([P, NKT, QG], fp8, tag=f"woh{mg}",
                                  name=f"woh{mg}")
                nc.sync.dma_start(wh_t[:], woh[:, :, msl])
                wl_t = wo_sb.tile([P, NKT, QG], fp8, tag=f"wol{mg}",
                                  name=f"wol{mg}")
                nc.sync.dma_start(wl_t[:], wol[:, :, msl])
                wo_t.append((wh_t, wl_t))

            # ------------- tail: remaining attention + out-projection ----
            def _proj(h, mg, csl_):
                width = csl_.stop - csl_.start
                wh_t, wl_t = wo_t[mg]
                psr = ps_proj.tile([P, QG], f32, tag="pq", name="pq")
                n = 0
                for st, mv in ((oh_sb[h], wh_t), (ol_sb[h], wh_t),
                               (oh_sb[h], wl_t)):
                    for u in range(NKT // 2):
                        us = slice(2 * u, 2 * u + 2)
                        nc.tensor.matmul(
                            psr[:, 0:width], st[:, us], mv[:, us, csl_],
                            start=(n == 0), stop=(n == 23),
                            perf_mode=DR)
                        n += 1
                rs = r_sb.tile([P, QG], f32, tag="rs", name="rs")
                nc.scalar.activation(rs[:, 0:width], psr[:, 0:width],
                                     COPY, scale=1.0 / 16384.0)
                msl = slice(mg * QG + csl_.start,
                            mg * QG + csl_.stop)
                nc.sync.dma_start(out[h * P:(h + 1) * P, msl],
                                  rs[:, 0:width])

            proj_jobs = [(h, mg) for h in range(HPC) for mg in range(NQG)]
            while ready:
                _drain(1)
            for i, (h, mg) in enumerate(proj_jobs):
                if i == len(proj_jobs) - 1:
                    # split the last group so the final copy+store tail
                    # is as short as possible
                    _proj(h, mg, slice(0, QG // 2))
                    _proj(h, mg, slice(QG // 2, 3 * QG // 4))
                    _proj(h, mg, slice(3 * QG // 4, QG))
                else:
                    _proj(h, mg, slice(0, QG))

    nc.compile()
    return nc


def _classify_mask(maskT):
    """Per (g, kt) block of maskT [k, q]: (SKIP|NOMASK|CAUSAL|MASKED, off)."""
    kinds = []
    qi = np.arange(QG)
    ki = np.arange(P)
    for g in range(NQG):
        row = []
        for kt in range(NKT):
            blk = maskT[kt * P:(kt + 1) * P, g * QG:(g + 1) * QG]
            if np.all(blk <= -1e5):
                row.append((SKIP, 0))
                continue
            if not blk.any():
                row.append((NOMASK, 0))
                continue
            c = kt - 4 * g
            if 0 <= c <= 3:
                keep = (g * QG + qi[None, :]) >= (kt * P + ki[:, None])
                if (np.all((blk == 0) == keep)
                        and np.all(blk[~keep] <= -1e5)):
                    row.append((CAUSAL, P * c))
                    continue
            off = 0
            while (off + P <= QG - P
                   and np.all(blk[:, off:off + P] <= -1e5)):
                off += P
            row.append((MASKED, off))
        kinds.append(tuple(row))
    return tuple(kinds)


def _get_nc(block_kind):
    key = ("nc", block_kind)
    if key not in _CACHE:
        _CACHE[key] = _build(block_kind)
    return _CACHE[key]


def _prep_inputs(x, freqs_cos, freqs_sin, mask, wq, wk, wv, wo):
    import ml_dtypes
    f = np.float32
    E4 = ml_dtypes.float8_e4m3
    BF = ml_dtypes.bfloat16

    x = np.asarray(x, f).reshape(S, D)
    mask = np.asarray(mask, f).reshape(S, S)
    wq, wk, wv, wo = (np.asarray(w, f) for w in (wq, wk, wv, wo))
    cos = np.asarray(freqs_cos, f)
    sin = np.asarray(freqs_sin, f)

    def split8(a):
        hi = a.astype(E4)
        lo = (a - hi.astype(f)).astype(E4)
        return hi, lo

    def to_pts(a2d, scale):
        """[R, C] f32 -> hi/lo fp8 in [P, R//P, C] layout."""
        a = a2d * scale
        hi, lo = split8(a)
        def lay(t):
            return np.ascontiguousarray(
                t.reshape(R // P, P, -1).transpose(1, 0, 2))
        R = a2d.shape[0]
        return lay(hi), lay(lo)

    xT = np.ascontiguousarray(x.T)                     # [D, S]
    xh_, xl_ = to_pts(xT, 16.0)

    # quadrant-interleaved rope layout for q/k head dims: quadrant q's
    # 32 slots hold even dims 2*(16q+t) for t<16, then their odd partners.
    # The t<->t+16 swap is then a DVE stream_shuffle (intra-quadrant).
    perm = np.empty(DH, np.int64)
    r = np.arange(DH)
    quad, t = r // 32, r % 32
    perm[:] = np.where(t < 16, 32 * quad + 2 * (t % 16),
                       32 * quad + 2 * (t % 16) + 1)
    jidx = 16 * quad + (t % 16)        # freq index per row
    maskT = np.ascontiguousarray(mask.T)

    C2 = cos.T[jidx]                                   # [128, S]
    S2 = np.where((t < 16)[:, None], -sin.T[jidx], sin.T[jidx])
    sc_q = (1.0 / 16384.0) / math.sqrt(DH)
    sc_k = 1.0 / 16384.0

    woT = np.ascontiguousarray(wo.T)                   # [j, m]
    woh_, wol_ = to_pts(woT, 1024.0)

    common = {
        "xh": xh_, "xl": xl_, "maskT": maskT,
        "woh": woh_, "wol": wol_,
        "cq": np.ascontiguousarray(C2 * sc_q).astype(BF),
        "sq": np.ascontiguousarray(S2 * sc_q).astype(BF),
        "ck": np.ascontiguousarray(C2 * sc_k).astype(BF),
        "sk": np.ascontiguousarray(S2 * sc_k).astype(BF),
        # sums moving vector: folds the 2^10 fp8-headroom scale so the
        # reciprocal of the raw PSUM sums is directly the oh/ol scale
        "ones": np.full((P, 1), 1024.0, BF),
        "cmask": np.broadcast_to(
            (np.arange(QG)[None, None, :]
             >= np.arange(P)[:, None, None]), (P, HPC, QG)).astype(BF),
    }
    in_maps = []
    for c in range(N_CORES):
        rows = np.arange(c * NH, (c + 1) * NH)
        # permuted row order for q/k: per head [evens; odds]
        rows_pm = np.concatenate(
            [c * NH + h * DH + perm for h in range(HPC)])
        m = dict(common)
        for kind, w in (("q", wq), ("k", wk), ("v", wv)):
            r = rows_pm if kind in ("q", "k") else rows
            wT = np.ascontiguousarray(w[r].T)          # [D, NH]
            hi, lo = to_pts(wT, 1024.0)                # [P, NDT, NH]
            if kind != "v":
                # per-head-major: [HPC, P, NDT, P]
                hi = np.ascontiguousarray(
                    hi.reshape(P, NDT, HPC, P).transpose(2, 0, 1, 3))
                lo = np.ascontiguousarray(
                    lo.reshape(P, NDT, HPC, P).transpose(2, 0, 1, 3))
            m[f"w{kind}h"] = hi
            m[f"w{kind}l"] = lo
        in_maps.append(m)
    return in_maps


def kernel(x, freqs_cos, freqs_sin, mask, wq, wk, wv, wo, start_pos):
    from concourse.bass_utils import run_bass_kernel_spmd

    in_maps = _prep_inputs(x, freqs_cos, freqs_sin, mask, wq, wk, wv, wo)
    nc = _get_nc(_classify_mask(in_maps[0]["maskT"]))
    res = run_bass_kernel_spmd(nc, in_maps, core_ids=list(range(N_CORES)))
    full = np.concatenate([res.results[c]["out"] for c in range(N_CORES)],
                          axis=0)
    return full.reshape(1, S, D).astype(np.float32)

